# revision 1
# baseline (speedup 1.0000x reference)
"""Trainium2 Bass kernel v2 for the MLP flow-matching GNN.

Strategy (8 cores, SPMD, uniform instruction stream):
  - Table rows p-major per core: row = c*6272 + (n%128)*49 + n//128.
  - Edges split by src-table half (lo: rows<25088, hi: rest) so dma_gather
    int16 indices fit; each half dst-sorted, packed into 512-edge blocks
    with <=128-node dst span; block counts padded to a common max.
  - Per group of G=8 blocks: one 4096-idx transpose dma_gather (src feats,
    feature-major), one 1024-idx dma_gather (H1d block rows), one 1024-idx
    dma_scatter_add (agg accumulation into DRAM).
  - Per block: m1^T = H1d_blk @ ST + W1c4 @ rel4 + I @ gsT (PSUM), SiLU,
    m2 = m1s^T chunks @ W2 + ones @ b2 (K=1), SiLU, agg = S^T-chunks @ m2s.
  - Node phase: feature-major matmuls, biases via act-bias / K=1 matmuls,
    LN via sum/sumsq accumulators, projected tables kept in SBUF; one
    AllGather of src-projections per layer.
"""

import os
import numpy as np
import ml_dtypes

BF16 = ml_dtypes.bfloat16
V2_LAYERS = int(os.environ.get("V2_LAYERS", "0")) or None   # debug: limit layers
V2_SKIP_CC = bool(int(os.environ.get("V2_SKIP_CC", "0")))   # debug: skip AllGather
V2_SKIP_EDGE = bool(int(os.environ.get("V2_SKIP_EDGE", "0")))
V2_SKIP_GATH = bool(int(os.environ.get("V2_SKIP_GATH", "0")))
V2_SIM_SAFE = bool(int(os.environ.get("V2_SIM_SAFE", "0")))  # Silu->Identity for CoreSim
V2_QSPREAD = bool(int(os.environ.get("V2_QSPREAD", "1")))   # spread SWDGE queues
V2_H1DSW = bool(int(os.environ.get("V2_H1DSW", "0")))       # h1d via dma_gather
V2_B2OLD = bool(int(os.environ.get("V2_B2OLD", "1")))       # per-chunk b2 matmuls
EPS = 1e-5
NCORES = 8
P = 128
BLK_E = 512
G = 8                 # blocks per stream group
V, E, L, H = 50000, 800000, 4, 128
NV = V // NCORES      # 6250
NW = (NV + P - 1) // P            # 49
NVP = NW * P                      # 6272 padded rows per core
HALF = 4 * NVP                    # 25088 rows per table half
DUMP = NVP                        # dump row base (rows NVP..NVP+127 are zeros)


def _silu(x):
    return x * (1.0 / (1.0 + np.exp(-x)))


def _remap(n):
    """local node id -> p-major row index."""
    return (n % P) * NW + n // P


def _wrap16(idx_flat):
    """Pack flat int16 indices i -> [16, n/16] at [i%16, i//16], tiled to 128."""
    n = idx_flat.shape[0]
    a = idx_flat.reshape(n // 16, 16).T.astype(np.int16)   # [16, n/16]
    return np.tile(a, (8, 1))


def _pack_half(dst_loc, src_row, rel, half_off):
    """Pack one half's dst-sorted edges into 512-edge, <=128-span blocks."""
    ec = dst_loc.shape[0]
    blocks = []
    e0 = 0
    while e0 < ec:
        base = int(dst_loc[e0])
        lim = int(np.searchsorted(dst_loc, base + P, side="left"))
        e1 = min(e0 + BLK_E, lim)
        blocks.append((e0, e1 - e0, base))
        e0 = e1
    return blocks


def _preprocess(inputs):
    pos0 = np.asarray(inputs["pos0"], np.float32)
    pos1 = np.asarray(inputs["pos1"], np.float32)
    z = np.asarray(inputs["z"], np.float32)
    t = np.asarray(inputs["t"], np.float32)
    edge_index = np.asarray(inputs["edge_index"])
    batch = np.asarray(inputs["batch"])
    ew1 = np.asarray(inputs["ew1"], np.float32)
    eb1 = np.asarray(inputs["eb1"], np.float32)
    ew2 = np.asarray(inputs["ew2"], np.float32)
    eb2 = np.asarray(inputs["eb2"], np.float32)
    nw1 = np.asarray(inputs["nw1"], np.float32)
    nb1 = np.asarray(inputs["nb1"], np.float32)
    nw2 = np.asarray(inputs["nw2"], np.float32)
    nb2 = np.asarray(inputs["nb2"], np.float32)
    ln_g = np.asarray(inputs["ln_g"], np.float32)
    ln_b = np.asarray(inputs["ln_b"], np.float32)
    te_w1 = np.asarray(inputs["te_w1"], np.float32)
    te_b1 = np.asarray(inputs["te_b1"], np.float32)
    te_w2 = np.asarray(inputs["te_w2"], np.float32)
    te_b2 = np.asarray(inputs["te_b2"], np.float32)
    cp_w = np.asarray(inputs["cp_w"], np.float32)
    cp_b = np.asarray(inputs["cp_b"], np.float32)

    ts = float(t[0])
    x_t = (1.0 - ts) * pos0 + ts * pos1
    target = pos1 - pos0

    t_emb = _silu(np.array([[ts]], np.float32) @ te_w1 + te_b1) @ te_w2 + te_b2
    h0 = np.concatenate(
        [z[batch], np.broadcast_to(t_emb, (V, t_emb.shape[1]))], axis=1
    ) @ cp_w + cp_b

    trivial_ln = bool(np.allclose(ln_g, 1.0) and np.allclose(ln_b, 0.0))

    # global node id -> table row
    gids = np.arange(V, dtype=np.int64)
    g2row = (gids // NV) * NVP + _remap(gids % NV)

    # layer-0 projected tables
    H1d0 = (h0 @ ew1[0, :H]).astype(np.float32)
    H1s0 = (h0 @ ew1[0, H:2 * H]).astype(np.float32)
    h1full0 = np.zeros((NCORES * NVP, H), np.float32)
    h1full0[g2row] = H1s0

    # dst-sorted edges, per-core ranges
    src_g = edge_index[0].astype(np.int64)
    dst_g = edge_index[1].astype(np.int64)
    order = np.argsort(dst_g, kind="stable")
    dst_s = dst_g[order]
    src_s = src_g[order]
    bounds = np.searchsorted(dst_s, np.arange(0, V + 1, NV))
    rel_all = (x_t[dst_s] - x_t[src_s]).astype(np.float32)
    srow_all = g2row[src_s]

    cores_raw = []
    nblk_lo_max = nblk_hi_max = 0
    for c in range(NCORES):
        e0, e1 = int(bounds[c]), int(bounds[c + 1])
        dl = (dst_s[e0:e1] - c * NV).astype(np.int64)
        sr = srow_all[e0:e1]
        rl = rel_all[e0:e1]
        is_lo = sr < HALF
        halves = []
        for hsel, off in ((is_lo, 0), (~is_lo, HALF)):
            d_h, s_h, r_h = dl[hsel], sr[hsel] - off, rl[hsel]
            blocks = _pack_half(d_h, s_h, r_h, off)
            halves.append((d_h, s_h, r_h, blocks))
        cores_raw.append(halves)
        nblk_lo_max = max(nblk_lo_max, len(halves[0][3]))
        nblk_hi_max = max(nblk_hi_max, len(halves[1][3]))

    ng_lo = (nblk_lo_max + G - 1) // G
    ng_hi = (nblk_hi_max + G - 1) // G
    NBLK_LO, NBLK_HI = ng_lo * G, ng_hi * G
    NBLK = NBLK_LO + NBLK_HI
    NG = ng_lo + ng_hi

    per_core = []
    slot_ar = np.arange(P, dtype=np.int64)
    for c in range(NCORES):
        dstrel = np.full(NBLK * BLK_E, -1, np.int16)
        srcidx = np.zeros(NBLK * BLK_E, np.int16)
        rel4 = np.zeros((4, NBLK * BLK_E), np.float32)
        scat = np.zeros((NBLK, P), np.int16)
        h1di = np.zeros((NBLK, P), np.int16)
        for hi, boff in ((0, 0), (1, NBLK_LO)):
            d_h, s_h, r_h, blocks = cores_raw[c][hi]
            for bi, (be0, bec, base) in enumerate(blocks):
                b = boff + bi
                sl = slice(b * BLK_E, b * BLK_E + bec)
                dstrel[sl] = (d_h[be0:be0 + bec] - base).astype(np.int16)
                srcidx[sl] = s_h[be0:be0 + bec].astype(np.int16)
                rel4[:3, sl] = r_h[be0:be0 + bec].T
                rel4[3, sl] = 1.0
                nblk_slots = base + slot_ar
                ok = nblk_slots < NV
                rows = np.where(ok, _remap(np.minimum(nblk_slots, NV - 1)),
                                DUMP + slot_ar)
                scat[b] = rows.astype(np.int16)
                h1di[b] = rows.astype(np.int16)
            # padding blocks: leave srcidx 0 / dstrel -1 / rel 0; route to dump
            for b in range(boff + len(blocks), boff + (NBLK_LO if hi == 0 else NBLK_HI)):
                scat[b] = (DUMP + slot_ar).astype(np.int16)
                h1di[b] = (DUMP + slot_ar).astype(np.int16)

        # indicator ST[slot, e] per block, streamed
        st = (dstrel.reshape(NBLK * BLK_E)[None, :]
              == np.arange(P, dtype=np.int16)[:, None]).astype(BF16)
        # group-wrapped gather/scatter indices
        sidx_w = np.concatenate(
            [_wrap16(srcidx[g * G * BLK_E:(g + 1) * G * BLK_E]) for g in range(NG)],
            axis=1)                                             # [128, NG*256]
        h1d_w = h1di.reshape(NBLK, P).T.astype(np.int32).copy()  # [128, NBLK]
        h1d16_w = np.concatenate(
            [_wrap16(h1di[g * G:(g + 1) * G].reshape(-1)) for g in range(NG)],
            axis=1)
        scat_w = np.concatenate(
            [_wrap16(scat[g * G:(g + 1) * G].reshape(-1)) for g in range(NG)],
            axis=1)                                             # [128, NG*64]

        nloc = c * NV + np.arange(NV)
        Hpad = np.zeros((NVP, H), np.float32)
        Hpad[:NV] = h0[nloc]
        hbuf0 = Hpad.reshape(NW, P, H).transpose(1, 0, 2).reshape(P, NVP)
        hT0 = Hpad.reshape(NW, P, H).transpose(2, 0, 1).reshape(H, NVP)
        h1d0 = np.zeros((NVP + P, H), np.float32)
        h1d0[_remap(np.arange(NV))] = H1d0[nloc]

        per_core.append(dict(
            dstrel=dstrel.reshape(NBLK * 4, P).T.copy(),     # [128, NBLK*4]
            srcidx16=sidx_w, h1didx32=h1d_w, h1didx16=h1d16_w, scatidx16=scat_w,
            st=st.astype(BF16), rel4=rel4.astype(BF16),
            hbuf0=hbuf0.astype(BF16), hT0=hT0.astype(BF16),
            h1d0=h1d0.astype(BF16),
        ))

    # weights, layer-concat layouts
    w1c4 = np.concatenate(
        [np.concatenate([ew1[l, 2 * H:], eb1[l][None, :]], 0) for l in range(L)],
        axis=1).astype(BF16)
    w1a = np.concatenate([ew1[l, :H] for l in range(L)], 1).astype(BF16)
    w1b = np.concatenate([ew1[l, H:2 * H] for l in range(L)], 1).astype(BF16)
    w2 = np.concatenate([ew2[l] for l in range(L)], 1).astype(BF16)
    b2row = eb2.reshape(1, L * H).astype(BF16)
    b2bc = np.concatenate(
        [np.broadcast_to(eb2[l], (P, H)) for l in range(L)], 1).astype(np.float32)
    nb2bc = np.concatenate(
        [np.broadcast_to(nb2[l], (P, H)) for l in range(L)], 1).astype(np.float32)
    b2row4 = np.concatenate([np.tile(eb2[l], 4) for l in range(L)]).reshape(1, L * 4 * H).astype(BF16)
    nw1h = np.concatenate([nw1[l, :H] for l in range(L)], 1).astype(BF16)
    nw1a = np.concatenate([nw1[l, H:] for l in range(L)], 1).astype(BF16)
    nw2c = np.concatenate([nw2[l] for l in range(L)], 1).astype(BF16)
    nb1c = nb1.T.astype(np.float32).copy()
    nb2row = nb2.reshape(1, L * H).astype(BF16)
    lngbc = np.concatenate(
        [np.broadcast_to(ln_g[l], (P, H)) for l in range(L)], 1).astype(np.float32)
    lnbbc = np.concatenate(
        [np.broadcast_to(ln_b[l], (P, H)) for l in range(L)], 1).astype(np.float32)
    ident = np.eye(P, dtype=BF16)
    iota16 = np.tile(np.arange(P, dtype=np.int16), (P, 1))
    onesr = np.ones((1, P), dtype=BF16)

    geom = dict(NBLK_LO=NBLK_LO, NBLK_HI=NBLK_HI, NG_LO=ng_lo, NG_HI=ng_hi,
                trivial_ln=trivial_ln)
    weights = dict(w1c4=w1c4, w1a=w1a, w1b=w1b, w2=w2, b2row=b2row, b2row4=b2row4, b2bc=b2bc, nb2bc=nb2bc,
                   nw1h=nw1h, nw1a=nw1a, nw2=nw2c, nb1c=nb1c, nb2row=nb2row,
                   lngbc=lngbc, lnbbc=lnbbc, ident=ident, iota16=iota16,
                   onesr=onesr)
    host = dict(h1full0=h1full0.astype(BF16), target=target,
                op_w=np.asarray(inputs["op_w"], np.float32),
                op_b=np.asarray(inputs["op_b"], np.float32))
    return geom, per_core, weights, host


def _build_program(geom):
    import concourse.bass as bass
    import concourse.bacc as bacc
    import concourse.mybir as mybir
    import concourse.tile as tile

    dt = mybir.dt
    AF = mybir.ActivationFunctionType
    AF_SILU = AF.Identity if V2_SIM_SAFE else AF.Silu
    ALU = mybir.AluOpType

    NBLK_LO, NBLK_HI = geom["NBLK_LO"], geom["NBLK_HI"]
    NG_LO, NG_HI = geom["NG_LO"], geom["NG_HI"]
    NBLK = NBLK_LO + NBLK_HI
    NG = NG_LO + NG_HI
    trivial_ln = geom["trivial_ln"]
    TROW = NCORES * NVP

    nc = bacc.Bacc(num_devices=NCORES, num_swdge_queues=4)

    # ---- parameters ----
    h1full0_d = nc.declare_dram_parameter("h1full0", [TROW, P], dt.bfloat16, isOutput=False)
    h1d0_d = nc.declare_dram_parameter("h1d0", [NVP + P, P], dt.bfloat16, isOutput=False)
    hbuf0_d = nc.declare_dram_parameter("hbuf0", [P, NVP], dt.bfloat16, isOutput=False)
    hT0_d = nc.declare_dram_parameter("hT0", [P, NVP], dt.bfloat16, isOutput=False)
    dstrel_d = nc.declare_dram_parameter("dstrel", [P, NBLK * 4], dt.int16, isOutput=False)
    srcidx_d = nc.declare_dram_parameter("srcidx16", [P, NG * 256], dt.int16, isOutput=False)
    h1didx_d = nc.declare_dram_parameter("h1didx32", [P, NBLK], dt.int32, isOutput=False)
    h1didx16_d = nc.declare_dram_parameter("h1didx16", [P, NG * 64], dt.int16, isOutput=False)
    scatidx_d = nc.declare_dram_parameter("scatidx16", [P, NG * 64], dt.int16, isOutput=False)
    st_d = nc.declare_dram_parameter("st", [P, NBLK * BLK_E], dt.bfloat16, isOutput=False)
    rel4_d = nc.declare_dram_parameter("rel4", [4, NBLK * BLK_E], dt.bfloat16, isOutput=False)
    w1c4_d = nc.declare_dram_parameter("w1c4", [4, L * P], dt.bfloat16, isOutput=False)
    w1a_d = nc.declare_dram_parameter("w1a", [P, L * P], dt.bfloat16, isOutput=False)
    w1b_d = nc.declare_dram_parameter("w1b", [P, L * P], dt.bfloat16, isOutput=False)
    w2_d = nc.declare_dram_parameter("w2", [P, L * P], dt.bfloat16, isOutput=False)
    b2row_d = nc.declare_dram_parameter("b2row", [1, L * P], dt.bfloat16, isOutput=False)
    b2row4_d = nc.declare_dram_parameter("b2row4", [1, L * 4 * P], dt.bfloat16, isOutput=False)
    b2bc_d = nc.declare_dram_parameter("b2bc", [P, L * P], dt.float32, isOutput=False)
    nb2bc_d = nc.declare_dram_parameter("nb2bc", [P, L * P], dt.float32, isOutput=False)
    nw1h_d = nc.declare_dram_parameter("nw1h", [P, L * P], dt.bfloat16, isOutput=False)
    nw1a_d = nc.declare_dram_parameter("nw1a", [P, L * P], dt.bfloat16, isOutput=False)
    nw2_d = nc.declare_dram_parameter("nw2", [P, L * P], dt.bfloat16, isOutput=False)
    nb1c_d = nc.declare_dram_parameter("nb1c", [P, L], dt.float32, isOutput=False)
    nb2row_d = nc.declare_dram_parameter("nb2row", [1, L * P], dt.bfloat16, isOutput=False)
    lngbc_d = nc.declare_dram_parameter("lngbc", [P, L * P], dt.float32, isOutput=False)
    lnbbc_d = nc.declare_dram_parameter("lnbbc", [P, L * P], dt.float32, isOutput=False)
    ident_d = nc.declare_dram_parameter("ident", [P, P], dt.bfloat16, isOutput=False)
    iota16_d = nc.declare_dram_parameter("iota16", [P, P], dt.int16, isOutput=False)
    onesr_d = nc.declare_dram_parameter("onesr", [1, P], dt.bfloat16, isOutput=False)
    hout_d = nc.declare_dram_parameter("hout", [P, NVP], dt.bfloat16, isOutput=True)

    # ---- internal DRAM ----
    h1d_hbm = nc.dram_tensor("h1d_hbm", [NVP + P, P], dt.bfloat16)
    agg_hbm = nc.dram_tensor("agg_hbm", [NVP + P, P], dt.bfloat16)
    h1sown = nc.dram_tensor("h1sown", [NVP, P], dt.bfloat16)
    table_hi = nc.dram_tensor("table_hi", [HALF, P], dt.bfloat16)
    h1full = [nc.dram_tensor(f"h1full{l}", [TROW, P], dt.bfloat16,
                             addr_space="Shared") for l in range(1, L)]

    groups = [list(range(NCORES))]

    with tile.TileContext(nc) as tc:
        with (
            tc.tile_pool(name="const", bufs=1) as cpool,
            tc.tile_pool(name="gst", bufs=2) as gpool,      # gsT stream
            tc.tile_pool(name="stm", bufs=2) as stpool,     # ST stream
            tc.tile_pool(name="hdb", bufs=2) as hdpool,     # h1d blocks
            tc.tile_pool(name="rel", bufs=2) as rpool,
            tc.tile_pool(name="wrk", bufs=3) as wpool,      # m1s/m2s/S
            tc.tile_pool(name="ast", bufs=2) as apool,      # agg staging
            tc.tile_pool(name="nod", bufs=3) as npool,      # node tiles
            tc.tile_pool(name="pm1", bufs=2, space="PSUM") as pm1,
            tc.tile_pool(name="pm2", bufs=2, space="PSUM") as pm2,
            tc.tile_pool(name="pms", bufs=2, space="PSUM") as pms,  # small psum
        ):
            def cload(src, shape, dtype, tag):
                t_ = cpool.tile(shape, dtype, tag=tag)
                nc.sync.dma_start(out=t_[:], in_=src[:, :])
                return t_

            identsb = cload(ident_d, [P, P], dt.bfloat16, "ident")
            iotasb = cload(iota16_d, [P, P], dt.int16, "iota")
            onesb = cload(onesr_d, [1, P], dt.bfloat16, "ones")
            dstrelsb = cload(dstrel_d, [P, NBLK * 4], dt.int16, "dstrel")
            srcidxsb = cload(srcidx_d, [P, NG * 256], dt.int16, "srcidx")
            h1didxsb = cload(h1didx_d, [P, NBLK], dt.int32, "h1didx")
            if V2_H1DSW:
                h1didx16sb = cload(h1didx16_d, [P, NG * 64], dt.int16, "h1didx16")
            scatidxsb = cload(scatidx_d, [P, NG * 64], dt.int16, "scatidx")
            w1c4sb = cload(w1c4_d, [4, L * P], dt.bfloat16, "w1c4")
            w1asb = cload(w1a_d, [P, L * P], dt.bfloat16, "w1a")
            w1bsb = cload(w1b_d, [P, L * P], dt.bfloat16, "w1b")
            w2sb = cload(w2_d, [P, L * P], dt.bfloat16, "w2")
            b2rowsb = cload(b2row_d, [1, L * P], dt.bfloat16, "b2row")
            b2bcsb = cload(b2bc_d, [P, L * P], dt.float32, "b2bc")
            nb2bcsb = cload(nb2bc_d, [P, L * P], dt.float32, "nb2bc")
            nw1hsb = cload(nw1h_d, [P, L * P], dt.bfloat16, "nw1h")
            nw1asb = cload(nw1a_d, [P, L * P], dt.bfloat16, "nw1a")
            nw2sb = cload(nw2_d, [P, L * P], dt.bfloat16, "nw2")
            nb1csb = cload(nb1c_d, [P, L], dt.float32, "nb1c")
            nb2rowsb = cload(nb2row_d, [1, L * P], dt.bfloat16, "nb2row")
            if not trivial_ln:
                lngbcsb = cload(lngbc_d, [P, L * P], dt.float32, "lngbc")
                lnbbcsb = cload(lnbbc_d, [P, L * P], dt.float32, "lnbbc")

            # persistent node-state buffers (SBUF)
            hbuf = cpool.tile([P, NVP], dt.bfloat16, tag="hbuf")
            hTbuf = cpool.tile([P, NVP], dt.bfloat16, tag="hTbuf")
            h1dbuf = cpool.tile([P, NVP], dt.bfloat16, tag="h1dbuf")
            h1sbuf = cpool.tile([P, NVP], dt.bfloat16, tag="h1sbuf")
            aggbuf = cpool.tile([P, NVP], dt.bfloat16, tag="aggbuf")
            xbuf = cpool.tile([P, NVP], dt.bfloat16, tag="xbuf")
            zeros = cpool.tile([P, NVP + P], dt.bfloat16, tag="zeros")
            sumbuf = cpool.tile([P, NW], dt.float32, tag="sumbuf")
            sqbuf = cpool.tile([P, NW], dt.float32, tag="sqbuf")
            mubuf = cpool.tile([P, NW], dt.float32, tag="mubuf")
            varbuf = cpool.tile([P, NW], dt.float32, tag="varbuf")
            sdbuf = cpool.tile([P, NW], dt.float32, tag="sdbuf")
            rstdb = cpool.tile([P, NW], dt.float32, tag="rstdb")
            msbuf = cpool.tile([P, NW], dt.float32, tag="msbuf")

            nc.sync.dma_start(out=hbuf[:], in_=hbuf0_d[:, :])
            nc.sync.dma_start(out=hTbuf[:], in_=hT0_d[:, :])
            nc.vector.memset(zeros[:], 0.0)
            nc.vector.memset(sumbuf[:], 0.0)
            nc.vector.memset(sqbuf[:], 1.0)
            nc.vector.memset(h1dbuf[:], 0.0)
            nc.vector.memset(h1sbuf[:], 0.0)
            # zero the dump rows of h1d_hbm once
            nc.sync.dma_start(out=h1d_hbm[NVP:NVP + P, :], in_=zeros[:, :P])

            def edge_phase(l, table_d, h1dsrc):
                lsl = slice(l * P, (l + 1) * P)
                # hi half of the gather table into its own tensor (dma_gather
                # silently ignores row offsets on in_ap)
                nc.sync.dma_start(out=table_hi[:, :], in_=table_d[HALF:, :])
                # zero agg (real + dump rows)
                nc.sync.dma_start(
                    out=agg_hbm[:, :].rearrange("(p w) f -> p w f", p=P),
                    in_=zeros[:].rearrange("p (w f) -> p w f", f=P))
                def load_group(g):
                    is_lo = g < NG_LO
                    tview = table_d[:, :] if is_lo else table_hi[:, :]
                    gsT = gpool.tile([P, G * BLK_E], dt.bfloat16, tag="gsT")
                    h1db = hdpool.tile([P, G * P], dt.bfloat16, tag="h1db")
                    nc.gpsimd.dma_gather(
                        out_ap=gsT[:].rearrange("p (c n) -> p c n", c=1),
                        in_ap=tview,
                        idxs_ap=srcidxsb[:, g * 256:(g + 1) * 256],
                        num_idxs=G * BLK_E, num_idxs_reg=G * BLK_E,
                        elem_size=P, transpose=True, single_packet=False,
                        queue_num=(g % 4) if V2_QSPREAD else 0)
                    for j in range(G):
                        nc.gpsimd.indirect_dma_start(
                            out=h1db[:, j * P:(j + 1) * P],
                            out_offset=None,
                            in_=h1dsrc[:, :],
                            in_offset=bass.IndirectOffsetOnAxis(
                                ap=h1didxsb[:, g * G + j:g * G + j + 1],
                                axis=0))
                    st = stpool.tile([P, G * BLK_E], dt.bfloat16, tag="st")
                    nc.sync.dma_start(
                        out=st[:], in_=st_d[:, g * G * BLK_E:(g + 1) * G * BLK_E])
                    r4 = rpool.tile([4, G * BLK_E], dt.bfloat16, tag="r4")
                    nc.sync.dma_start(
                        out=r4[:], in_=rel4_d[:, g * G * BLK_E:(g + 1) * G * BLK_E])
                    return gsT, h1db, st, r4

                tiles = load_group(0)
                for g in range(NG):
                    gsT, h1db, st, r4 = tiles
                    nxt = load_group(g + 1) if g + 1 < NG else None
                    aggst = apool.tile([P, G * P], dt.bfloat16, tag="aggst")

                    for j in range(G):
                        b = g * G + j
                        esl = slice(j * BLK_E, (j + 1) * BLK_E)
                        m1p = pm1.tile([P, BLK_E], dt.float32, tag="m1")
                        nc.tensor.matmul(m1p[:], lhsT=h1db[:, j * P:(j + 1) * P],
                                         rhs=st[:, esl], start=True, stop=False,
                                         skip_group_check=True)
                        nc.tensor.matmul(m1p[:], lhsT=w1c4sb[:, lsl],
                                         rhs=r4[:, esl], start=False, stop=False,
                                         skip_group_check=True)
                        nc.tensor.matmul(m1p[:], lhsT=identsb[:],
                                         rhs=gsT[:, esl], start=False, stop=True,
                                         skip_group_check=True)
                        m1s = wpool.tile([P, BLK_E], dt.bfloat16, tag="m1s")
                        nc.scalar.activation(m1s[:], m1p[:], AF_SILU)

                        m2p = pm2.tile([P, BLK_E], dt.float32, tag="m2")
                        for k in range(4):
                            ksl = slice(k * P, (k + 1) * P)
                            nc.tensor.matmul(m2p[:, ksl], lhsT=m1s[:, ksl],
                                             rhs=w2sb[:, lsl], start=True,
                                             stop=True, skip_group_check=True)
                        nc.vector.tensor_tensor(
                            out=m2p[:].rearrange("p (k f) -> p k f", f=P),
                            in0=m2p[:].rearrange("p (k f) -> p k f", f=P),
                            in1=b2bcsb[:, lsl].unsqueeze(1).to_broadcast([P, 4, P]),
                            op=ALU.add)
                        m2s = wpool.tile([P, BLK_E], dt.bfloat16, tag="m2s")
                        nc.scalar.activation(m2s[:], m2p[:], AF_SILU)

                        S = wpool.tile([P, 4, P], dt.bfloat16, tag="S")
                        nc.vector.tensor_tensor(
                            out=S[:],
                            in0=dstrelsb[:, 4 * b:4 * b + 4].unsqueeze(2)
                                .to_broadcast([P, 4, P]),
                            in1=iotasb[:].unsqueeze(1).to_broadcast([P, 4, P]),
                            op=ALU.is_equal)
                        aggp = pms.tile([P, P], dt.float32, tag="psA")
                        for k in range(4):
                            nc.tensor.matmul(aggp[:], lhsT=S[:, k, :],
                                             rhs=m2s[:, k * P:(k + 1) * P],
                                             start=(k == 0), stop=(k == 3))
                        nc.vector.tensor_copy(aggst[:, j * P:(j + 1) * P], aggp[:])

                    nc.gpsimd.dma_scatter_add(
                        out_ap=agg_hbm[:, :],
                        in_ap=aggst[:].rearrange("p (c n) -> p c n", n=P),
                        idxs_ap=scatidxsb[:, g * 64:(g + 1) * 64],
                        num_idxs=G * P, num_idxs_reg=G * P, elem_size=P,
                        single_packet=False, queue_num=2)
                    tiles = nxt

            def node_phase(l):
                lsl = slice(l * P, (l + 1) * P)
                last = l == L - 1
                nc.sync.dma_start(
                    out=aggbuf[:].rearrange("p (w f) -> p w f", f=P),
                    in_=agg_hbm[:NVP, :].rearrange("(p w) f -> p w f", p=P))
                for w in range(NW):
                    cnt = min(P, NV - w * P)
                    wsl = slice(w * P, w * P + P)
                    csl = slice(w * P, w * P + cnt)
                    tp = pms.tile([P, P], dt.bfloat16, tag="psB")
                    nc.tensor.transpose(tp[:, :cnt], aggbuf[:cnt, wsl],
                                        identsb[:cnt, :cnt])
                    aggt = npool.tile([P, P], dt.bfloat16, tag="aggt")
                    nc.vector.tensor_copy(aggt[:, :cnt], tp[:, :cnt])
                    n1p = pms.tile([P, P], dt.float32, tag="psA")
                    nc.tensor.matmul(n1p[:, :cnt], lhsT=nw1hsb[:, lsl],
                                     rhs=hTbuf[:, csl], start=True, stop=False)
                    nc.tensor.matmul(n1p[:, :cnt], lhsT=nw1asb[:, lsl],
                                     rhs=aggt[:, :cnt], start=False, stop=True)
                    n1s = npool.tile([P, P], dt.bfloat16, tag="n1s")
                    nc.scalar.activation(n1s[:, :cnt], n1p[:, :cnt], AF_SILU,
                                         bias=nb1csb[:, l:l + 1])
                    n2p = pms.tile([P, P], dt.float32, tag="psB")
                    nc.tensor.matmul(n2p[:cnt, :], lhsT=n1s[:, :cnt],
                                     rhs=nw2sb[:, lsl], start=True, stop=True)
                    nc.vector.tensor_tensor(out=n2p[:cnt, :], in0=n2p[:cnt, :],
                                            in1=nb2bcsb[:cnt, lsl], op=ALU.add)
                    # x = n2 + h (residual); Act does copy+sum, square+sumsq
                    nc.vector.tensor_tensor(out=n2p[:cnt, :], in0=n2p[:cnt, :],
                                            in1=hbuf[:cnt, wsl], op=ALU.add)
                    nc.scalar.activation(xbuf[:cnt, wsl], n2p[:cnt, :], AF.Copy,
                                         accum_out=sumbuf[:cnt, w:w + 1])
                    sqs = npool.tile([P, P], dt.float32, tag="sqs")
                    nc.scalar.activation(sqs[:cnt, :], n2p[:cnt, :], AF.Square,
                                         accum_out=sqbuf[:cnt, w:w + 1])

                # batched LN stats
                nc.vector.tensor_scalar_mul(mubuf[:], sumbuf[:], 1.0 / P)
                nc.vector.tensor_tensor(out=varbuf[:], in0=mubuf[:],
                                        in1=mubuf[:], op=ALU.mult)
                nc.vector.tensor_scalar(sqbuf[:], sqbuf[:], 1.0 / P, EPS,
                                        op0=ALU.mult, op1=ALU.add)
                nc.vector.tensor_tensor(out=varbuf[:], in0=sqbuf[:],
                                        in1=varbuf[:], op=ALU.subtract)
                nc.scalar.activation(sdbuf[:], varbuf[:], AF.Sqrt)
                nc.vector.reciprocal(rstdb[:], sdbuf[:])
                nc.vector.tensor_tensor(out=msbuf[:], in0=mubuf[:],
                                        in1=rstdb[:], op=ALU.mult)

                for w in range(NW):
                    cnt = min(P, NV - w * P)
                    wsl = slice(w * P, w * P + P)
                    csl = slice(w * P, w * P + cnt)
                    nsl = slice((l + 1) * P, (l + 2) * P)
                    if trivial_ln:
                        nc.vector.tensor_scalar(
                            hbuf[:cnt, wsl], xbuf[:cnt, wsl],
                            rstdb[:cnt, w:w + 1], msbuf[:cnt, w:w + 1],
                            op0=ALU.mult, op1=ALU.subtract)
                    else:
                        xn = npool.tile([P, P], dt.float32, tag="xn")
                        nc.vector.tensor_scalar(
                            xn[:cnt, :], xbuf[:cnt, wsl],
                            rstdb[:cnt, w:w + 1], msbuf[:cnt, w:w + 1],
                            op0=ALU.mult, op1=ALU.subtract)
                        nc.vector.tensor_tensor(out=xn[:cnt, :], in0=xn[:cnt, :],
                                                in1=lngbcsb[:cnt, lsl], op=ALU.mult)
                        nc.vector.tensor_tensor(out=hbuf[:cnt, wsl], in0=xn[:cnt, :],
                                                in1=lnbbcsb[:cnt, lsl], op=ALU.add)
                    if not last:
                        tp2 = pms.tile([P, P], dt.bfloat16, tag="psB")
                        nc.tensor.transpose(tp2[:, :cnt], hbuf[:cnt, wsl],
                                            identsb[:cnt, :cnt])
                        nc.vector.tensor_copy(hTbuf[:, csl], tp2[:, :cnt])
                        pd = pms.tile([P, P], dt.float32, tag="psA")
                        nc.tensor.matmul(pd[:cnt, :], lhsT=hTbuf[:, csl],
                                         rhs=w1asb[:, nsl], start=True, stop=True)
                        nc.vector.tensor_copy(h1dbuf[:cnt, wsl], pd[:cnt, :])
                        ps = pms.tile([P, P], dt.float32, tag="psB")
                        nc.tensor.matmul(ps[:cnt, :], lhsT=hTbuf[:, csl],
                                         rhs=w1bsb[:, nsl], start=True, stop=True)
                        nc.vector.tensor_copy(h1sbuf[:cnt, wsl], ps[:cnt, :])

                if last:
                    nc.sync.dma_start(out=hout_d[:, :], in_=hbuf[:])
                else:
                    nc.sync.dma_start(
                        out=h1d_hbm[:NVP, :].rearrange("(p w) f -> p w f", p=P),
                        in_=h1dbuf[:].rearrange("p (w f) -> p w f", f=P))
                    nc.sync.dma_start(
                        out=h1sown[:, :].rearrange("(p w) f -> p (w f)", p=P),
                        in_=h1sbuf[:])
                    if not V2_SKIP_CC:
                        nc.gpsimd.collective_compute(
                            "AllGather", mybir.AluOpType.bypass,
                            replica_groups=groups,
                            ins=[h1sown[:, :]],
                            outs=[h1full[l][:, :]])

            for l in range(V2_LAYERS or L):
                table = h1full0_d if l == 0 else h1full[l - 1]
                h1dsrc = h1d0_d if l == 0 else h1d_hbm
                if not V2_SKIP_EDGE:
                    edge_phase(l, table, h1dsrc)
                node_phase(l)

    nc.finalize()
    return nc


def _make_in_maps(geom, per_core, weights, host):
    in_maps = []
    for c in range(NCORES):
        pc = per_core[c]
        m = {
            "h1full0": host["h1full0"],
            "h1d0": pc["h1d0"], "hbuf0": pc["hbuf0"], "hT0": pc["hT0"],
            "dstrel": pc["dstrel"], "srcidx16": pc["srcidx16"],
            "h1didx32": pc["h1didx32"], "h1didx16": pc["h1didx16"],
            "scatidx16": pc["scatidx16"],
            "st": pc["st"], "rel4": pc["rel4"],
        }
        m.update(weights)
        in_maps.append(m)
    return in_maps


def _postprocess(host, houts):
    hs = []
    for c in range(NCORES):
        hp = houts[c].astype(np.float32)              # [128, NVP] p-major
        hn = hp.reshape(P, NW, P).transpose(1, 0, 2).reshape(NVP, P)
        hs.append(hn[:NV])
    h = np.concatenate(hs, axis=0)
    v_pred = h @ host["op_w"] + host["op_b"]
    diff = v_pred - host["target"]
    return np.float32(np.mean(diff.astype(np.float64) ** 2))


def kernel(**inputs):
    from concourse.bass_utils import run_bass_kernel_spmd

    geom, per_core, weights, host = _preprocess(inputs)
    nc = _build_program(geom)
    in_maps = _make_in_maps(geom, per_core, weights, host)
    res = run_bass_kernel_spmd(nc, in_maps, list(range(NCORES)))
    houts = [res.results[c]["hout"] for c in range(NCORES)]
    return _postprocess(host, houts)



# revision 18
# speedup vs baseline: 1.2343x; 1.2343x over previous
"""Trainium2 Bass kernel v3 for the MLP flow-matching GNN.

Strategy (8 cores, SPMD, uniform instruction stream):
  - Table rows p-major per core: row = c*6272 + (n%128)*49 + n//128.
  - Edges split by src-table half (lo: rows<25088, hi: rest) so dma_gather
    int16 indices fit; each half dst-sorted, packed into 1024-edge
    SUPER-blocks with <=128-node dst span (2 sub-blocks of 512 each);
    super-block counts padded to a common max across cores, pad blocks
    carry srcidx=-1 so the Q7 ucode trims their descriptor generation.
  - Per group of G=4 super-blocks: one 4096-idx transpose dma_gather (src
    feats, feature-major).  Per HALF: one 128*NSB-idx non-transpose
    dma_gather for all h1d rows, and one dma_scatter_add for all agg rows
    (batching kills the per-block SWDGE descriptor-generation cost that
    dominated v2).
  - Per sub-block: m1^T = h1d_sb @ ST + W1c4 @ rel4 + I @ gsT (PSUM), SiLU,
    m2 = m1s^T chunks @ W2 (K=128), +b2, SiLU; agg chunks accumulate into a
    single [128,128] PSUM tile per super-block, staged to aggst.
  - Node phase: feature-major matmuls, biases via act-bias, LN via sum/sumsq
    accumulators, projected tables kept in SBUF; one AllGather of src
    projections per layer.
"""

import os
import numpy as np
import ml_dtypes

BF16 = ml_dtypes.bfloat16
V2_LAYERS = int(os.environ.get("V2_LAYERS", "0")) or None   # debug: limit layers
V2_SKIP_CC = bool(int(os.environ.get("V2_SKIP_CC", "0")))   # debug: skip AllGather
V2_SIM_SAFE = bool(int(os.environ.get("V2_SIM_SAFE", "0")))  # Silu->Identity for CoreSim
V2_SINGLE_PACKET = bool(int(os.environ.get("V2_SINGLE_PACKET", "0")))
V3_TRIM = bool(int(os.environ.get("V3_TRIM", "0")))  # -1 pads: Q7 trims desc-gen
EPS = 1e-5
NCORES = 8
P = 128
BLK_E = 512           # sub-block (m1/m2 tile) edge count
SB_E = 1024           # super-block edge count (shares h1d rows + agg rows)
SUB = SB_E // BLK_E   # sub-blocks per super-block
G = 4                 # super-blocks per gather group (G*SB_E = 4096 idxs)
V, E, L, H = 50000, 800000, 4, 128
NV = V // NCORES      # 6250
NW = (NV + P - 1) // P            # 49
NVP = NW * P                      # 6272 padded rows per core
HALF = 4 * NVP                    # 25088 rows per table half
DUMP = NVP                        # dump row base (rows NVP..NVP+127 are zeros)


def _silu(x):
    return x * (1.0 / (1.0 + np.exp(-x)))


def _remap(n):
    """local node id -> p-major row index."""
    return (n % P) * NW + n // P


def _wrap16(idx_flat):
    """Pack flat int16 indices i -> [16, n/16] at [i%16, i//16], tiled to 128."""
    n = idx_flat.shape[0]
    a = idx_flat.reshape(n // 16, 16).T.astype(np.int16)   # [16, n/16]
    return np.tile(a, (8, 1))


def _pack_half(dst_loc):
    """Pack one half's dst-sorted edges into <=1024-edge, <=128-span blocks."""
    ec = dst_loc.shape[0]
    blocks = []
    e0 = 0
    while e0 < ec:
        base = int(dst_loc[e0])
        lim = int(np.searchsorted(dst_loc, base + P, side="left"))
        e1 = min(e0 + SB_E, lim)
        blocks.append((e0, e1 - e0, base))
        e0 = e1
    return blocks


def _preprocess(inputs):
    pos0 = np.asarray(inputs["pos0"], np.float32)
    pos1 = np.asarray(inputs["pos1"], np.float32)
    z = np.asarray(inputs["z"], np.float32)
    t = np.asarray(inputs["t"], np.float32)
    edge_index = np.asarray(inputs["edge_index"])
    batch = np.asarray(inputs["batch"])
    ew1 = np.asarray(inputs["ew1"], np.float32)
    eb1 = np.asarray(inputs["eb1"], np.float32)
    ew2 = np.asarray(inputs["ew2"], np.float32)
    eb2 = np.asarray(inputs["eb2"], np.float32)
    nw1 = np.asarray(inputs["nw1"], np.float32)
    nb1 = np.asarray(inputs["nb1"], np.float32)
    nw2 = np.asarray(inputs["nw2"], np.float32)
    nb2 = np.asarray(inputs["nb2"], np.float32)
    ln_g = np.asarray(inputs["ln_g"], np.float32)
    ln_b = np.asarray(inputs["ln_b"], np.float32)
    te_w1 = np.asarray(inputs["te_w1"], np.float32)
    te_b1 = np.asarray(inputs["te_b1"], np.float32)
    te_w2 = np.asarray(inputs["te_w2"], np.float32)
    te_b2 = np.asarray(inputs["te_b2"], np.float32)
    cp_w = np.asarray(inputs["cp_w"], np.float32)
    cp_b = np.asarray(inputs["cp_b"], np.float32)

    ts = float(t[0])
    x_t = (1.0 - ts) * pos0 + ts * pos1
    target = pos1 - pos0

    t_emb = _silu(np.array([[ts]], np.float32) @ te_w1 + te_b1) @ te_w2 + te_b2
    h0 = np.concatenate(
        [z[batch], np.broadcast_to(t_emb, (V, t_emb.shape[1]))], axis=1
    ) @ cp_w + cp_b

    trivial_ln = bool(np.allclose(ln_g, 1.0) and np.allclose(ln_b, 0.0))

    # global node id -> table row
    gids = np.arange(V, dtype=np.int64)
    g2row = (gids // NV) * NVP + _remap(gids % NV)

    # layer-0 projected tables
    H1d0 = (h0 @ ew1[0, :H]).astype(np.float32)
    H1s0 = (h0 @ ew1[0, H:2 * H]).astype(np.float32)
    h1full0 = np.zeros((NCORES * NVP, H), np.float32)
    h1full0[g2row] = H1s0

    # dst-sorted edges, per-core ranges
    src_g = edge_index[0].astype(np.int64)
    dst_g = edge_index[1].astype(np.int64)
    order = np.argsort(dst_g, kind="stable")
    dst_s = dst_g[order]
    src_s = src_g[order]
    bounds = np.searchsorted(dst_s, np.arange(0, V + 1, NV))
    rel_all = (x_t[dst_s] - x_t[src_s]).astype(np.float32)
    srow_all = g2row[src_s]

    cores_raw = []
    nsb_lo_max = nsb_hi_max = 0
    for c in range(NCORES):
        e0, e1 = int(bounds[c]), int(bounds[c + 1])
        dl = (dst_s[e0:e1] - c * NV).astype(np.int64)
        sr = srow_all[e0:e1]
        rl = rel_all[e0:e1]
        is_lo = sr < HALF
        halves = []
        for hsel, off in ((is_lo, 0), (~is_lo, HALF)):
            d_h, s_h, r_h = dl[hsel], sr[hsel] - off, rl[hsel]
            blocks = _pack_half(d_h)
            halves.append((d_h, s_h, r_h, blocks))
        cores_raw.append(halves)
        nsb_lo_max = max(nsb_lo_max, len(halves[0][3]))
        nsb_hi_max = max(nsb_hi_max, len(halves[1][3]))

    ng_lo = (nsb_lo_max + G - 1) // G
    ng_hi = (nsb_hi_max + G - 1) // G
    NSB_LO, NSB_HI = ng_lo * G, ng_hi * G
    NSB = NSB_LO + NSB_HI
    NG = ng_lo + ng_hi

    per_core = []
    slot_ar = np.arange(P, dtype=np.int64)
    for c in range(NCORES):
        padidx = -1 if V3_TRIM else 0
        dstrel = np.full(NSB * SB_E, -1, np.int16)
        srcidx = np.full(NSB * SB_E, padidx, np.int16)
        rel4 = np.zeros((4, NSB * SB_E), np.float32)
        scat = np.zeros((NSB, P), np.int16)
        h1di = np.zeros((NSB, P), np.int16)
        if V3_TRIM:
            scat[:] = -1
            h1di[:] = -1
        else:
            scat[:] = (DUMP + slot_ar).astype(np.int16)[None, :]
            h1di[:] = (DUMP + slot_ar).astype(np.int16)[None, :]
        for hi, boff, nsb_h in ((0, 0, NSB_LO), (1, NSB_LO, NSB_HI)):
            d_h, s_h, r_h, blocks = cores_raw[c][hi]
            for bi, (be0, bec, base) in enumerate(blocks):
                b = boff + bi
                sl = slice(b * SB_E, b * SB_E + bec)
                dstrel[sl] = (d_h[be0:be0 + bec] - base).astype(np.int16)
                srcidx[sl] = s_h[be0:be0 + bec].astype(np.int16)
                # dead slots inside a real block: gather row 0 (valid data)
                dd = slice(b * SB_E + bec, (b + 1) * SB_E)
                srcidx[dd] = 0
                rel4[:3, sl] = r_h[be0:be0 + bec].T
                rel4[3, sl] = 1.0
                nblk_slots = base + slot_ar
                ok = nblk_slots < NV
                rows = np.where(ok, _remap(np.minimum(nblk_slots, NV - 1)),
                                DUMP + slot_ar)
                scat[b] = rows.astype(np.int16)
                h1di[b] = rows.astype(np.int16)
            # pad blocks: srcidx/scat/h1di = -1 (trailing: ucode trims) if
            # V3_TRIM else row0/dump (always-valid addresses)

        # group-wrapped gather indices [128, NG*256]
        sidx_w = np.concatenate(
            [_wrap16(srcidx[g * G * SB_E:(g + 1) * G * SB_E]) for g in range(NG)],
            axis=1)
        # per-half h1d gather / scatter indices
        h1d_lo = _wrap16(h1di[:NSB_LO].reshape(-1))      # [128, NSB_LO*8]
        h1d_hi = _wrap16(h1di[NSB_LO:].reshape(-1))
        scat_lo = _wrap16(scat[:NSB_LO].reshape(-1))
        scat_hi = _wrap16(scat[NSB_LO:].reshape(-1))

        # indicator ST[slot, e], streamed per group
        st = (dstrel[None, :]
              == np.arange(P, dtype=np.int16)[:, None]).astype(BF16)

        nloc = c * NV + np.arange(NV)
        Hpad = np.zeros((NVP, H), np.float32)
        Hpad[:NV] = h0[nloc]
        hbuf0 = Hpad.reshape(NW, P, H).transpose(1, 0, 2).reshape(P, NVP)
        hT0 = Hpad.reshape(NW, P, H).transpose(2, 0, 1).reshape(H, NVP)
        h1d0 = np.zeros((NVP + P, H), np.float32)
        h1d0[_remap(np.arange(NV))] = H1d0[nloc]

        per_core.append(dict(
            dstrel=dstrel.reshape(NSB * SUB * 4, P).T.copy(),  # [128, NSB*8]
            srcidx16=sidx_w,
            h1dlo=h1d_lo, h1dhi=h1d_hi, scatlo=scat_lo, scathi=scat_hi,
            st=st.astype(BF16), rel4=rel4.astype(BF16),
            hbuf0=hbuf0.astype(BF16), hT0=hT0.astype(BF16),
            h1d0=h1d0.astype(BF16),
        ))

    # weights, layer-concat layouts
    w1c4 = np.concatenate(
        [np.concatenate([ew1[l, 2 * H:], eb1[l][None, :]], 0) for l in range(L)],
        axis=1).astype(BF16)
    w1a = np.concatenate([ew1[l, :H] for l in range(L)], 1).astype(BF16)
    w1b = np.concatenate([ew1[l, H:2 * H] for l in range(L)], 1).astype(BF16)
    w2 = np.concatenate([ew2[l] for l in range(L)], 1).astype(BF16)
    b2bc = np.concatenate(
        [np.broadcast_to(eb2[l], (P, H)) for l in range(L)], 1).astype(np.float32)
    nb2bc = np.concatenate(
        [np.broadcast_to(nb2[l], (P, H)) for l in range(L)], 1).astype(np.float32)
    nw1h = np.concatenate([nw1[l, :H] for l in range(L)], 1).astype(BF16)
    nw1a = np.concatenate([nw1[l, H:] for l in range(L)], 1).astype(BF16)
    nw2c = np.concatenate([nw2[l] for l in range(L)], 1).astype(BF16)
    nb1c = nb1.T.astype(np.float32).copy()
    lngbc = np.concatenate(
        [np.broadcast_to(ln_g[l], (P, H)) for l in range(L)], 1).astype(np.float32)
    lnbbc = np.concatenate(
        [np.broadcast_to(ln_b[l], (P, H)) for l in range(L)], 1).astype(np.float32)
    ident = np.eye(P, dtype=BF16)
    iota16 = np.tile(np.arange(P, dtype=np.int16), (P, 1))

    geom = dict(NSB_LO=NSB_LO, NSB_HI=NSB_HI, NG_LO=ng_lo, NG_HI=ng_hi,
                trivial_ln=trivial_ln)
    weights = dict(w1c4=w1c4, w1a=w1a, w1b=w1b, w2=w2, b2bc=b2bc, nb2bc=nb2bc,
                   nw1h=nw1h, nw1a=nw1a, nw2=nw2c, nb1c=nb1c,
                   lngbc=lngbc, lnbbc=lnbbc, ident=ident, iota16=iota16)
    host = dict(h1full0=h1full0.astype(BF16), target=target,
                op_w=np.asarray(inputs["op_w"], np.float32),
                op_b=np.asarray(inputs["op_b"], np.float32))
    return geom, per_core, weights, host


def _build_program(geom):
    import concourse.bass as bass
    import concourse.bacc as bacc
    import concourse.mybir as mybir
    import concourse.tile as tile

    dt = mybir.dt
    AF = mybir.ActivationFunctionType
    AF_SILU = AF.Identity if V2_SIM_SAFE else AF.Silu
    ALU = mybir.AluOpType

    NSB_LO, NSB_HI = geom["NSB_LO"], geom["NSB_HI"]
    NG_LO, NG_HI = geom["NG_LO"], geom["NG_HI"]
    NSB = NSB_LO + NSB_HI
    NG = NG_LO + NG_HI
    trivial_ln = geom["trivial_ln"]
    TROW = NCORES * NVP
    SP = V2_SINGLE_PACKET

    nc = bacc.Bacc(num_devices=NCORES, num_swdge_queues=4)

    # ---- parameters ----
    h1full0_d = nc.declare_dram_parameter("h1full0", [TROW, P], dt.bfloat16, isOutput=False)
    h1d0_d = nc.declare_dram_parameter("h1d0", [NVP + P, P], dt.bfloat16, isOutput=False)
    hbuf0_d = nc.declare_dram_parameter("hbuf0", [P, NVP], dt.bfloat16, isOutput=False)
    hT0_d = nc.declare_dram_parameter("hT0", [P, NVP], dt.bfloat16, isOutput=False)
    dstrel_d = nc.declare_dram_parameter("dstrel", [P, NSB * SUB * 4], dt.int16, isOutput=False)
    srcidx_d = nc.declare_dram_parameter("srcidx16", [P, NG * 256], dt.int16, isOutput=False)
    h1dlo_d = nc.declare_dram_parameter("h1dlo", [P, NSB_LO * 8], dt.int16, isOutput=False)
    h1dhi_d = nc.declare_dram_parameter("h1dhi", [P, NSB_HI * 8], dt.int16, isOutput=False)
    scatlo_d = nc.declare_dram_parameter("scatlo", [P, NSB_LO * 8], dt.int16, isOutput=False)
    scathi_d = nc.declare_dram_parameter("scathi", [P, NSB_HI * 8], dt.int16, isOutput=False)
    st_d = nc.declare_dram_parameter("st", [P, NSB * SB_E], dt.bfloat16, isOutput=False)
    rel4_d = nc.declare_dram_parameter("rel4", [4, NSB * SB_E], dt.bfloat16, isOutput=False)
    w1c4_d = nc.declare_dram_parameter("w1c4", [4, L * P], dt.bfloat16, isOutput=False)
    w1a_d = nc.declare_dram_parameter("w1a", [P, L * P], dt.bfloat16, isOutput=False)
    w1b_d = nc.declare_dram_parameter("w1b", [P, L * P], dt.bfloat16, isOutput=False)
    w2_d = nc.declare_dram_parameter("w2", [P, L * P], dt.bfloat16, isOutput=False)
    b2bc_d = nc.declare_dram_parameter("b2bc", [P, L * P], dt.float32, isOutput=False)
    nb2bc_d = nc.declare_dram_parameter("nb2bc", [P, L * P], dt.float32, isOutput=False)
    nw1h_d = nc.declare_dram_parameter("nw1h", [P, L * P], dt.bfloat16, isOutput=False)
    nw1a_d = nc.declare_dram_parameter("nw1a", [P, L * P], dt.bfloat16, isOutput=False)
    nw2_d = nc.declare_dram_parameter("nw2", [P, L * P], dt.bfloat16, isOutput=False)
    nb1c_d = nc.declare_dram_parameter("nb1c", [P, L], dt.float32, isOutput=False)
    lngbc_d = nc.declare_dram_parameter("lngbc", [P, L * P], dt.float32, isOutput=False)
    lnbbc_d = nc.declare_dram_parameter("lnbbc", [P, L * P], dt.float32, isOutput=False)
    ident_d = nc.declare_dram_parameter("ident", [P, P], dt.bfloat16, isOutput=False)
    iota16_d = nc.declare_dram_parameter("iota16", [P, P], dt.int16, isOutput=False)
    hout_d = nc.declare_dram_parameter("hout", [P, NVP], dt.bfloat16, isOutput=True)

    # ---- internal DRAM ----
    h1d_hbm = nc.dram_tensor("h1d_hbm", [NVP + P, P], dt.bfloat16)
    agg_hbm = nc.dram_tensor("agg_hbm", [NVP + P, P], dt.bfloat16)
    agg_zero = nc.dram_tensor("agg_zero", [NVP + P, P], dt.bfloat16)
    h1sown = nc.dram_tensor("h1sown", [NVP, P], dt.bfloat16)
    table_hi = nc.dram_tensor("table_hi", [HALF, P], dt.bfloat16)
    h1full = [nc.dram_tensor(f"h1full{l}", [TROW, P], dt.bfloat16,
                             addr_space="Shared") for l in range(1, L)]

    groups = [list(range(NCORES))]

    with tile.TileContext(nc) as tc:
        with (
            tc.tile_pool(name="const", bufs=1) as cpool,
            tc.tile_pool(name="gst", bufs=2) as gpool,      # gsT stream
            tc.tile_pool(name="sxp", bufs=2) as sxpool,     # srcidx stream
            tc.tile_pool(name="stm", bufs=2) as stpool,     # ST stream
            tc.tile_pool(name="hdb", bufs=1) as hdpool,     # h1d per-half tiles
            tc.tile_pool(name="rel", bufs=2) as rpool,
            tc.tile_pool(name="wrk", bufs=3) as wpool,      # m1s/m2s/S
            tc.tile_pool(name="ast", bufs=1) as apool,      # agg staging per half
            tc.tile_pool(name="nod", bufs=3) as npool,      # node tiles
            tc.tile_pool(name="pm1", bufs=2, space="PSUM") as pm1,
            tc.tile_pool(name="pm2", bufs=2, space="PSUM") as pm2,
            tc.tile_pool(name="pms", bufs=2, space="PSUM") as pms,  # small psum
        ):
            def cload(src, shape, dtype, tag):
                t_ = cpool.tile(shape, dtype, tag=tag)
                nc.sync.dma_start(out=t_[:], in_=src[:, :])
                return t_

            identsb = cload(ident_d, [P, P], dt.bfloat16, "ident")
            iotasb = cload(iota16_d, [P, P], dt.int16, "iota")
            dstrelsb = cload(dstrel_d, [P, NSB * SUB * 4], dt.int16, "dstrel")
            h1dlosb = cload(h1dlo_d, [P, NSB_LO * 8], dt.int16, "h1dlo")
            h1dhisb = cload(h1dhi_d, [P, NSB_HI * 8], dt.int16, "h1dhi")
            scatlosb = cload(scatlo_d, [P, NSB_LO * 8], dt.int16, "scatlo")
            scathisb = cload(scathi_d, [P, NSB_HI * 8], dt.int16, "scathi")
            w1c4sb = cload(w1c4_d, [4, L * P], dt.bfloat16, "w1c4")
            w1asb = cload(w1a_d, [P, L * P], dt.bfloat16, "w1a")
            w1bsb = cload(w1b_d, [P, L * P], dt.bfloat16, "w1b")
            w2sb = cload(w2_d, [P, L * P], dt.bfloat16, "w2")
            b2bcsb = cload(b2bc_d, [P, L * P], dt.float32, "b2bc")
            nb2bcsb = cload(nb2bc_d, [P, L * P], dt.float32, "nb2bc")
            nw1hsb = cload(nw1h_d, [P, L * P], dt.bfloat16, "nw1h")
            nw1asb = cload(nw1a_d, [P, L * P], dt.bfloat16, "nw1a")
            nw2sb = cload(nw2_d, [P, L * P], dt.bfloat16, "nw2")
            nb1csb = cload(nb1c_d, [P, L], dt.float32, "nb1c")
            if not trivial_ln:
                lngbcsb = cload(lngbc_d, [P, L * P], dt.float32, "lngbc")
                lnbbcsb = cload(lnbbc_d, [P, L * P], dt.float32, "lnbbc")

            # persistent node-state buffers (SBUF)
            hbuf = cpool.tile([P, NVP], dt.bfloat16, tag="hbuf")
            hTbuf = cpool.tile([P, NVP], dt.bfloat16, tag="hTbuf")
            h1dbuf = cpool.tile([P, NVP], dt.bfloat16, tag="h1dbuf")
            aggbuf = cpool.tile([P, NVP], dt.bfloat16, tag="aggbuf")
            xbuf = cpool.tile([P, NVP], dt.bfloat16, tag="xbuf")
            zeros = cpool.tile([P, P], dt.bfloat16, tag="zeros")
            sumbuf = cpool.tile([P, NW], dt.float32, tag="sumbuf")
            sqbuf = cpool.tile([P, NW], dt.float32, tag="sqbuf")
            mubuf = cpool.tile([P, NW], dt.float32, tag="mubuf")
            varbuf = cpool.tile([P, NW], dt.float32, tag="varbuf")
            sdbuf = cpool.tile([P, NW], dt.float32, tag="sdbuf")
            rstdb = cpool.tile([P, NW], dt.float32, tag="rstdb")
            msbuf = cpool.tile([P, NW], dt.float32, tag="msbuf")

            nc.sync.dma_start(out=hbuf[:], in_=hbuf0_d[:, :])
            nc.sync.dma_start(out=hTbuf[:], in_=hT0_d[:, :])
            nc.vector.memset(zeros[:], 0.0)
            nc.vector.memset(sumbuf[:], 0.0)
            nc.vector.memset(sqbuf[:], 1.0)
            nc.vector.memset(h1dbuf[:], 0.0)
            nc.vector.memset(xbuf[:], 0.0)
            # zero the dump rows of h1d_hbm + the agg_zero template once
            nc.sync.dma_start(out=h1d_hbm[NVP:NVP + P, :], in_=zeros[:, :])
            for zi in range(NW + 1):
                nc.sync.dma_start(out=agg_zero[zi * P:(zi + 1) * P, :],
                                  in_=zeros[:, :])

            def edge_phase(l, table_d, h1dsrc):
                lsl = slice(l * P, (l + 1) * P)
                # hi half of the gather table into its own tensor (dma_gather
                # silently ignores row offsets on in_ap)
                nc.sync.dma_start(out=table_hi[:, :], in_=table_d[HALF:, :])
                # zero agg (real + dump rows) via DRAM->DRAM template copy
                nc.sync.dma_start(out=agg_hbm[:, :], in_=agg_zero[:, :])

                # per-half h1d rows: one batched non-transpose gather each
                h1d_lo_t = hdpool.tile([P, NSB_LO * P], dt.bfloat16, tag="h1dlo_t")
                h1d_hi_t = hdpool.tile([P, NSB_HI * P], dt.bfloat16, tag="h1dhi_t")
                h1dall = {"lo": h1d_lo_t, "hi": h1d_hi_t}
                for hname, nsb_h, idxsb in (("lo", NSB_LO, h1dlosb),
                                            ("hi", NSB_HI, h1dhisb)):
                    nc.gpsimd.dma_gather(
                        out_ap=h1dall[hname][:].rearrange("p (c n) -> p c n", n=P),
                        in_ap=h1dsrc[:, :],
                        idxs_ap=idxsb[:, :],
                        num_idxs=nsb_h * P, num_idxs_reg=nsb_h * P,
                        elem_size=P, transpose=False, single_packet=SP,
                        queue_num=0)

                def load_group(g):
                    is_lo = g < NG_LO
                    tview = table_d[:, :] if is_lo else table_hi[:, :]
                    sidx = sxpool.tile([P, 256], dt.int16, tag="sidx")
                    nc.sync.dma_start(out=sidx[:],
                                      in_=srcidx_d[:, g * 256:(g + 1) * 256])
                    gsT = gpool.tile([P, G * SB_E], dt.bfloat16, tag="gsT")
                    nc.gpsimd.dma_gather(
                        out_ap=gsT[:].rearrange("p (c n) -> p c n", c=1),
                        in_ap=tview,
                        idxs_ap=sidx[:, :],
                        num_idxs=G * SB_E, num_idxs_reg=G * SB_E,
                        elem_size=P, transpose=True, single_packet=SP,
                        queue_num=0)
                    st = stpool.tile([P, G * SB_E], dt.bfloat16, tag="st")
                    nc.sync.dma_start(
                        out=st[:], in_=st_d[:, g * G * SB_E:(g + 1) * G * SB_E])
                    r4 = rpool.tile([4, G * SB_E], dt.bfloat16, tag="r4")
                    nc.sync.dma_start(
                        out=r4[:], in_=rel4_d[:, g * G * SB_E:(g + 1) * G * SB_E])
                    return gsT, st, r4

                aggst_lo = apool.tile([P, NSB_LO * P], dt.bfloat16, tag="astlo")
                aggst_hi = apool.tile([P, NSB_HI * P], dt.bfloat16, tag="asthi")
                aggst = {"lo": aggst_lo, "hi": aggst_hi}
                tiles = load_group(0)
                for g in range(NG):
                    gsT, st, r4 = tiles
                    nxt = load_group(g + 1) if g + 1 < NG else None
                    is_lo = g < NG_LO
                    hname = "lo" if is_lo else "hi"
                    h1dh = h1dall[hname]
                    boff = 0 if is_lo else NSB_LO
                    gh = g if is_lo else g - NG_LO   # group idx within half

                    for j in range(G):
                        sb = g * G + j                  # global super-block
                        sbh = gh * G + j                # super-block within half
                        aggp = pms.tile([P, P], dt.float32, tag="psA")
                        for u in range(SUB):
                            esl = slice((j * SUB + u) * BLK_E,
                                        (j * SUB + u + 1) * BLK_E)
                            m1p = pm1.tile([P, BLK_E], dt.float32, tag="m1")
                            nc.tensor.matmul(m1p[:], lhsT=h1dh[:, sbh * P:(sbh + 1) * P],
                                             rhs=st[:, esl], start=True, stop=False,
                                             skip_group_check=True)
                            nc.tensor.matmul(m1p[:], lhsT=w1c4sb[:, lsl],
                                             rhs=r4[:, esl], start=False, stop=False,
                                             skip_group_check=True)
                            nc.tensor.matmul(m1p[:], lhsT=identsb[:],
                                             rhs=gsT[:, esl], start=False, stop=True,
                                             skip_group_check=True)
                            m1s = wpool.tile([P, BLK_E], dt.bfloat16, tag="m1s")
                            nc.scalar.activation(m1s[:], m1p[:], AF_SILU)

                            m2p = pm2.tile([P, BLK_E], dt.float32, tag="m2")
                            for k in range(4):
                                ksl = slice(k * P, (k + 1) * P)
                                nc.tensor.matmul(m2p[:, ksl], lhsT=m1s[:, ksl],
                                                 rhs=w2sb[:, lsl], start=True,
                                                 stop=True, skip_group_check=True)
                            nc.vector.tensor_tensor(
                                out=m2p[:].rearrange("p (k f) -> p k f", f=P),
                                in0=m2p[:].rearrange("p (k f) -> p k f", f=P),
                                in1=b2bcsb[:, lsl].unsqueeze(1).to_broadcast([P, 4, P]),
                                op=ALU.add)
                            m2s = wpool.tile([P, BLK_E], dt.bfloat16, tag="m2s")
                            nc.scalar.activation(m2s[:], m2p[:], AF_SILU)

                            S = wpool.tile([P, 4, P], dt.bfloat16, tag="S")
                            bidx = sb * SUB + u
                            nc.vector.tensor_tensor(
                                out=S[:],
                                in0=dstrelsb[:, 4 * bidx:4 * bidx + 4].unsqueeze(2)
                                    .to_broadcast([P, 4, P]),
                                in1=iotasb[:].unsqueeze(1).to_broadcast([P, 4, P]),
                                op=ALU.is_equal)
                            for k in range(4):
                                nc.tensor.matmul(aggp[:], lhsT=S[:, k, :],
                                                 rhs=m2s[:, k * P:(k + 1) * P],
                                                 start=(u == 0 and k == 0),
                                                 stop=(u == SUB - 1 and k == 3),
                                                 skip_group_check=True)
                        nc.vector.tensor_copy(
                            aggst[hname][:, sbh * P:(sbh + 1) * P], aggp[:])

                    if (is_lo and g == NG_LO - 1) or g == NG - 1:
                        nsb_h = NSB_LO if is_lo else NSB_HI
                        scatsb = scatlosb if is_lo else scathisb
                        nc.gpsimd.dma_scatter_add(
                            out_ap=agg_hbm[:, :],
                            in_ap=aggst[hname][:].rearrange("p (c n) -> p c n", n=P),
                            idxs_ap=scatsb[:, :],
                            num_idxs=nsb_h * P, num_idxs_reg=nsb_h * P,
                            elem_size=P, single_packet=SP, queue_num=0)
                    tiles = nxt

            def node_phase(l):
                lsl = slice(l * P, (l + 1) * P)
                last = l == L - 1
                nc.sync.dma_start(
                    out=aggbuf[:].rearrange("p (w f) -> p w f", f=P),
                    in_=agg_hbm[:NVP, :].rearrange("(p w) f -> p w f", p=P))
                for w in range(NW):
                    cnt = min(P, NV - w * P)
                    wsl = slice(w * P, w * P + P)
                    csl = slice(w * P, w * P + cnt)
                    tp = pms.tile([P, P], dt.bfloat16, tag="psB")
                    nc.tensor.transpose(tp[:, :cnt], aggbuf[:cnt, wsl],
                                        identsb[:cnt, :cnt])
                    aggt = npool.tile([P, P], dt.bfloat16, tag="aggt")
                    nc.vector.tensor_copy(aggt[:, :cnt], tp[:, :cnt])
                    n1p = pms.tile([P, P], dt.float32, tag="psA")
                    nc.tensor.matmul(n1p[:, :cnt], lhsT=nw1hsb[:, lsl],
                                     rhs=hTbuf[:, csl], start=True, stop=False)
                    nc.tensor.matmul(n1p[:, :cnt], lhsT=nw1asb[:, lsl],
                                     rhs=aggt[:, :cnt], start=False, stop=True)
                    n1s = npool.tile([P, P], dt.bfloat16, tag="n1s")
                    nc.scalar.activation(n1s[:, :cnt], n1p[:, :cnt], AF_SILU,
                                         bias=nb1csb[:, l:l + 1])
                    n2p = pms.tile([P, P], dt.float32, tag="psB")
                    nc.tensor.matmul(n2p[:cnt, :], lhsT=n1s[:, :cnt],
                                     rhs=nw2sb[:, lsl], start=True, stop=True)
                    nc.vector.tensor_tensor(out=n2p[:cnt, :], in0=n2p[:cnt, :],
                                            in1=nb2bcsb[:cnt, lsl], op=ALU.add)
                    # x = n2 + h (residual); Act does copy+sum, square+sumsq
                    nc.vector.tensor_tensor(out=n2p[:cnt, :], in0=n2p[:cnt, :],
                                            in1=hbuf[:cnt, wsl], op=ALU.add)
                    nc.scalar.activation(xbuf[:cnt, wsl], n2p[:cnt, :], AF.Copy,
                                         accum_out=sumbuf[:cnt, w:w + 1])
                    sqs = npool.tile([P, P], dt.float32, tag="sqs")
                    nc.scalar.activation(sqs[:cnt, :], n2p[:cnt, :], AF.Square,
                                         accum_out=sqbuf[:cnt, w:w + 1])

                # batched LN stats
                nc.vector.tensor_scalar_mul(mubuf[:], sumbuf[:], 1.0 / P)
                nc.vector.tensor_tensor(out=varbuf[:], in0=mubuf[:],
                                        in1=mubuf[:], op=ALU.mult)
                nc.vector.tensor_scalar(sqbuf[:], sqbuf[:], 1.0 / P, EPS,
                                        op0=ALU.mult, op1=ALU.add)
                nc.vector.tensor_tensor(out=varbuf[:], in0=sqbuf[:],
                                        in1=varbuf[:], op=ALU.subtract)
                nc.scalar.activation(sdbuf[:], varbuf[:], AF.Sqrt)
                nc.vector.reciprocal(rstdb[:], sdbuf[:])
                nc.vector.tensor_tensor(out=msbuf[:], in0=mubuf[:],
                                        in1=rstdb[:], op=ALU.mult)

                for w in range(NW):
                    cnt = min(P, NV - w * P)
                    wsl = slice(w * P, w * P + P)
                    csl = slice(w * P, w * P + cnt)
                    nsl = slice((l + 1) * P, (l + 2) * P)
                    if trivial_ln:
                        nc.vector.tensor_scalar(
                            hbuf[:cnt, wsl], xbuf[:cnt, wsl],
                            rstdb[:cnt, w:w + 1], msbuf[:cnt, w:w + 1],
                            op0=ALU.mult, op1=ALU.subtract)
                    else:
                        xn = npool.tile([P, P], dt.float32, tag="xn")
                        nc.vector.tensor_scalar(
                            xn[:cnt, :], xbuf[:cnt, wsl],
                            rstdb[:cnt, w:w + 1], msbuf[:cnt, w:w + 1],
                            op0=ALU.mult, op1=ALU.subtract)
                        nc.vector.tensor_tensor(out=xn[:cnt, :], in0=xn[:cnt, :],
                                                in1=lngbcsb[:cnt, lsl], op=ALU.mult)
                        nc.vector.tensor_tensor(out=hbuf[:cnt, wsl], in0=xn[:cnt, :],
                                                in1=lnbbcsb[:cnt, lsl], op=ALU.add)
                    if not last:
                        tp2 = pms.tile([P, P], dt.bfloat16, tag="psB")
                        nc.tensor.transpose(tp2[:, :cnt], hbuf[:cnt, wsl],
                                            identsb[:cnt, :cnt])
                        nc.vector.tensor_copy(hTbuf[:, csl], tp2[:, :cnt])
                        pd = pms.tile([P, P], dt.float32, tag="psA")
                        nc.tensor.matmul(pd[:cnt, :], lhsT=hTbuf[:, csl],
                                         rhs=w1asb[:, nsl], start=True, stop=True)
                        nc.vector.tensor_copy(h1dbuf[:cnt, wsl], pd[:cnt, :])
                        ps = pms.tile([P, P], dt.float32, tag="psB")
                        nc.tensor.matmul(ps[:cnt, :], lhsT=hTbuf[:, csl],
                                         rhs=w1bsb[:, nsl], start=True, stop=True)
                        # xbuf[w] is dead after the LN read above: reuse as
                        # the h1s staging buffer to save SBUF
                        nc.vector.tensor_copy(xbuf[:cnt, wsl], ps[:cnt, :])

                if last:
                    nc.sync.dma_start(out=hout_d[:, :], in_=hbuf[:])
                else:
                    nc.sync.dma_start(
                        out=h1d_hbm[:NVP, :].rearrange("(p w) f -> p w f", p=P),
                        in_=h1dbuf[:].rearrange("p (w f) -> p w f", f=P))
                    nc.sync.dma_start(
                        out=h1sown[:, :].rearrange("(p w) f -> p (w f)", p=P),
                        in_=xbuf[:])
                    if not V2_SKIP_CC:
                        nc.gpsimd.collective_compute(
                            "AllGather", mybir.AluOpType.bypass,
                            replica_groups=groups,
                            ins=[h1sown[:, :]],
                            outs=[h1full[l][:, :]])

            for l in range(V2_LAYERS or L):
                table = h1full0_d if l == 0 else h1full[l - 1]
                h1dsrc = h1d0_d if l == 0 else h1d_hbm
                edge_phase(l, table, h1dsrc)
                node_phase(l)

    nc.finalize()
    return nc


def _make_in_maps(geom, per_core, weights, host):
    in_maps = []
    for c in range(NCORES):
        pc = per_core[c]
        m = {
            "h1full0": host["h1full0"],
            "h1d0": pc["h1d0"], "hbuf0": pc["hbuf0"], "hT0": pc["hT0"],
            "dstrel": pc["dstrel"], "srcidx16": pc["srcidx16"],
            "h1dlo": pc["h1dlo"], "h1dhi": pc["h1dhi"],
            "scatlo": pc["scatlo"], "scathi": pc["scathi"],
            "st": pc["st"], "rel4": pc["rel4"],
        }
        m.update(weights)
        in_maps.append(m)
    return in_maps


def _postprocess(host, houts):
    hs = []
    for c in range(NCORES):
        hp = houts[c].astype(np.float32)              # [128, NVP] p-major
        hn = hp.reshape(P, NW, P).transpose(1, 0, 2).reshape(NVP, P)
        hs.append(hn[:NV])
    h = np.concatenate(hs, axis=0)
    v_pred = h @ host["op_w"] + host["op_b"]
    diff = v_pred - host["target"]
    return np.float32(np.mean(diff.astype(np.float64) ** 2))


def kernel(**inputs):
    from concourse.bass_utils import run_bass_kernel_spmd

    geom, per_core, weights, host = _preprocess(inputs)
    nc = _build_program(geom)
    in_maps = _make_in_maps(geom, per_core, weights, host)
    res = run_bass_kernel_spmd(nc, in_maps, list(range(NCORES)))
    houts = [res.results[c]["hout"] for c in range(NCORES)]
    return _postprocess(host, houts)


# revision 20
# speedup vs baseline: 1.3052x; 1.0575x over previous
"""Trainium2 Bass kernel v3 for the MLP flow-matching GNN.

Strategy (8 cores, SPMD, uniform instruction stream):
  - Table rows p-major per core: row = c*6272 + (n%128)*49 + n//128.
  - Edges split by src-table half (lo: rows<25088, hi: rest) so dma_gather
    int16 indices fit; each half dst-sorted, packed into 1024-edge
    SUPER-blocks with <=128-node dst span (2 sub-blocks of 512 each);
    super-block counts padded to a common max across cores, pad blocks
    carry srcidx=-1 so the Q7 ucode trims their descriptor generation.
  - Per group of G=4 super-blocks: one 4096-idx transpose dma_gather (src
    feats, feature-major).  Per HALF: one 128*NSB-idx non-transpose
    dma_gather for all h1d rows, and one dma_scatter_add for all agg rows
    (batching kills the per-block SWDGE descriptor-generation cost that
    dominated v2).
  - Per sub-block: m1^T = h1d_sb @ ST + W1c4 @ rel4 + I @ gsT (PSUM), SiLU,
    m2 = m1s^T chunks @ W2 (K=128), +b2, SiLU; agg chunks accumulate into a
    single [128,128] PSUM tile per super-block, staged to aggst.
  - Node phase: feature-major matmuls, biases via act-bias, LN via sum/sumsq
    accumulators, projected tables kept in SBUF; one AllGather of src
    projections per layer.
"""

import os
import numpy as np
import ml_dtypes

BF16 = ml_dtypes.bfloat16
V2_LAYERS = int(os.environ.get("V2_LAYERS", "0")) or None   # debug: limit layers
V2_SKIP_CC = bool(int(os.environ.get("V2_SKIP_CC", "0")))   # debug: skip AllGather
V2_SIM_SAFE = bool(int(os.environ.get("V2_SIM_SAFE", "0")))  # Silu->Identity for CoreSim
V2_SINGLE_PACKET = bool(int(os.environ.get("V2_SINGLE_PACKET", "0")))
V3_TRIM = bool(int(os.environ.get("V3_TRIM", "0")))  # -1 pads: Q7 trims desc-gen
EPS = 1e-5
NCORES = 8
P = 128
BLK_E = 512           # sub-block (m1/m2 tile) edge count
SB_E = 1024           # super-block edge count (shares h1d rows + agg rows)
SUB = SB_E // BLK_E   # sub-blocks per super-block
G = 4                 # super-blocks per gather group (G*SB_E = 4096 idxs)
V, E, L, H = 50000, 800000, 4, 128
NV = V // NCORES      # 6250
NW = (NV + P - 1) // P            # 49
NVP = NW * P                      # 6272 padded rows per core
HALF = 4 * NVP                    # 25088 rows per table half
DUMP = NVP                        # dump row base (rows NVP..NVP+127 are zeros)


def _silu(x):
    return x * (1.0 / (1.0 + np.exp(-x)))


def _remap(n):
    """local node id -> p-major row index."""
    return (n % P) * NW + n // P


def _wrap16(idx_flat):
    """Pack flat int16 indices i -> [16, n/16] at [i%16, i//16], tiled to 128."""
    n = idx_flat.shape[0]
    a = idx_flat.reshape(n // 16, 16).T.astype(np.int16)   # [16, n/16]
    return np.tile(a, (8, 1))


def _pack_half(dst_loc):
    """Pack one half's dst-sorted edges into <=1024-edge, <=128-span blocks."""
    ec = dst_loc.shape[0]
    blocks = []
    e0 = 0
    while e0 < ec:
        base = int(dst_loc[e0])
        lim = int(np.searchsorted(dst_loc, base + P, side="left"))
        e1 = min(e0 + SB_E, lim)
        blocks.append((e0, e1 - e0, base))
        e0 = e1
    return blocks


def _preprocess(inputs):
    pos0 = np.asarray(inputs["pos0"], np.float32)
    pos1 = np.asarray(inputs["pos1"], np.float32)
    z = np.asarray(inputs["z"], np.float32)
    t = np.asarray(inputs["t"], np.float32)
    edge_index = np.asarray(inputs["edge_index"])
    batch = np.asarray(inputs["batch"])
    ew1 = np.asarray(inputs["ew1"], np.float32)
    eb1 = np.asarray(inputs["eb1"], np.float32)
    ew2 = np.asarray(inputs["ew2"], np.float32)
    eb2 = np.asarray(inputs["eb2"], np.float32)
    nw1 = np.asarray(inputs["nw1"], np.float32)
    nb1 = np.asarray(inputs["nb1"], np.float32)
    nw2 = np.asarray(inputs["nw2"], np.float32)
    nb2 = np.asarray(inputs["nb2"], np.float32)
    ln_g = np.asarray(inputs["ln_g"], np.float32)
    ln_b = np.asarray(inputs["ln_b"], np.float32)
    te_w1 = np.asarray(inputs["te_w1"], np.float32)
    te_b1 = np.asarray(inputs["te_b1"], np.float32)
    te_w2 = np.asarray(inputs["te_w2"], np.float32)
    te_b2 = np.asarray(inputs["te_b2"], np.float32)
    cp_w = np.asarray(inputs["cp_w"], np.float32)
    cp_b = np.asarray(inputs["cp_b"], np.float32)

    ts = float(t[0])
    x_t = (1.0 - ts) * pos0 + ts * pos1
    target = pos1 - pos0

    t_emb = _silu(np.array([[ts]], np.float32) @ te_w1 + te_b1) @ te_w2 + te_b2
    h0 = np.concatenate(
        [z[batch], np.broadcast_to(t_emb, (V, t_emb.shape[1]))], axis=1
    ) @ cp_w + cp_b

    trivial_ln = bool(np.allclose(ln_g, 1.0) and np.allclose(ln_b, 0.0))

    # global node id -> table row
    gids = np.arange(V, dtype=np.int64)
    g2row = (gids // NV) * NVP + _remap(gids % NV)

    # layer-0 projected tables
    H1d0 = (h0 @ ew1[0, :H]).astype(np.float32)
    H1s0 = (h0 @ ew1[0, H:2 * H]).astype(np.float32)
    h1full0 = np.zeros((NCORES * NVP, H), np.float32)
    h1full0[g2row] = H1s0

    # dst-sorted edges, per-core ranges
    src_g = edge_index[0].astype(np.int64)
    dst_g = edge_index[1].astype(np.int64)
    order = np.argsort(dst_g, kind="stable")
    dst_s = dst_g[order]
    src_s = src_g[order]
    bounds = np.searchsorted(dst_s, np.arange(0, V + 1, NV))
    rel_all = (x_t[dst_s] - x_t[src_s]).astype(np.float32)
    srow_all = g2row[src_s]

    cores_raw = []
    nsb_lo_max = nsb_hi_max = 0
    for c in range(NCORES):
        e0, e1 = int(bounds[c]), int(bounds[c + 1])
        dl = (dst_s[e0:e1] - c * NV).astype(np.int64)
        sr = srow_all[e0:e1]
        rl = rel_all[e0:e1]
        is_lo = sr < HALF
        halves = []
        for hsel, off in ((is_lo, 0), (~is_lo, HALF)):
            d_h, s_h, r_h = dl[hsel], sr[hsel] - off, rl[hsel]
            blocks = _pack_half(d_h)
            halves.append((d_h, s_h, r_h, blocks))
        cores_raw.append(halves)
        nsb_lo_max = max(nsb_lo_max, len(halves[0][3]))
        nsb_hi_max = max(nsb_hi_max, len(halves[1][3]))

    ng_lo = (nsb_lo_max + G - 1) // G
    ng_hi = (nsb_hi_max + G - 1) // G
    NSB_LO, NSB_HI = ng_lo * G, ng_hi * G
    NSB = NSB_LO + NSB_HI
    NG = ng_lo + ng_hi

    per_core = []
    slot_ar = np.arange(P, dtype=np.int64)
    for c in range(NCORES):
        padidx = -1 if V3_TRIM else 0
        dstrel = np.full(NSB * SB_E, -1, np.int16)
        srcidx = np.full(NSB * SB_E, padidx, np.int16)
        rel4 = np.zeros((4, NSB * SB_E), np.float32)
        scat = np.zeros((NSB, P), np.int16)
        h1di = np.zeros((NSB, P), np.int16)
        if V3_TRIM:
            scat[:] = -1
            h1di[:] = -1
        else:
            scat[:] = (DUMP + slot_ar).astype(np.int16)[None, :]
            h1di[:] = (DUMP + slot_ar).astype(np.int16)[None, :]
        for hi, boff, nsb_h in ((0, 0, NSB_LO), (1, NSB_LO, NSB_HI)):
            d_h, s_h, r_h, blocks = cores_raw[c][hi]
            for bi, (be0, bec, base) in enumerate(blocks):
                b = boff + bi
                sl = slice(b * SB_E, b * SB_E + bec)
                dstrel[sl] = (d_h[be0:be0 + bec] - base).astype(np.int16)
                srcidx[sl] = s_h[be0:be0 + bec].astype(np.int16)
                # dead slots inside a real block: gather row 0 (valid data)
                dd = slice(b * SB_E + bec, (b + 1) * SB_E)
                srcidx[dd] = 0
                rel4[:3, sl] = r_h[be0:be0 + bec].T
                rel4[3, sl] = 1.0
                nblk_slots = base + slot_ar
                ok = nblk_slots < NV
                rows = np.where(ok, _remap(np.minimum(nblk_slots, NV - 1)),
                                DUMP + slot_ar)
                scat[b] = rows.astype(np.int16)
                h1di[b] = rows.astype(np.int16)
            # pad blocks: srcidx/scat/h1di = -1 (trailing: ucode trims) if
            # V3_TRIM else row0/dump (always-valid addresses)

        # group-wrapped gather indices [128, NG*256]
        sidx_w = np.concatenate(
            [_wrap16(srcidx[g * G * SB_E:(g + 1) * G * SB_E]) for g in range(NG)],
            axis=1)
        # per-half h1d gather / scatter indices
        h1d_lo = _wrap16(h1di[:NSB_LO].reshape(-1))      # [128, NSB_LO*8]
        h1d_hi = _wrap16(h1di[NSB_LO:].reshape(-1))
        scat_lo = _wrap16(scat[:NSB_LO].reshape(-1))
        scat_hi = _wrap16(scat[NSB_LO:].reshape(-1))

        # indicator ST[slot, e], streamed per group
        st = (dstrel[None, :]
              == np.arange(P, dtype=np.int16)[:, None]).astype(BF16)

        nloc = c * NV + np.arange(NV)
        Hpad = np.zeros((NVP, H), np.float32)
        Hpad[:NV] = h0[nloc]
        hbuf0 = Hpad.reshape(NW, P, H).transpose(1, 0, 2).reshape(P, NVP)
        hT0 = Hpad.reshape(NW, P, H).transpose(2, 0, 1).reshape(H, NVP)
        h1d0 = np.zeros((NVP + P, H), np.float32)
        h1d0[_remap(np.arange(NV))] = H1d0[nloc]

        per_core.append(dict(
            dstrel=dstrel.reshape(NSB * SUB * 4, P).T.copy(),  # [128, NSB*8]
            srcidx16=sidx_w,
            h1dlo=h1d_lo, h1dhi=h1d_hi, scatlo=scat_lo, scathi=scat_hi,
            st=st.astype(BF16), rel4=rel4.astype(BF16),
            hbuf0=hbuf0.astype(BF16), hT0=hT0.astype(BF16),
            h1d0=h1d0.astype(BF16),
        ))

    # weights, layer-concat layouts
    w1c4 = np.concatenate(
        [np.concatenate([ew1[l, 2 * H:], eb1[l][None, :]], 0) for l in range(L)],
        axis=1).astype(BF16)
    w1a = np.concatenate([ew1[l, :H] for l in range(L)], 1).astype(BF16)
    w1b = np.concatenate([ew1[l, H:2 * H] for l in range(L)], 1).astype(BF16)
    w2 = np.concatenate([ew2[l] for l in range(L)], 1).astype(BF16)
    b2bc = np.concatenate(
        [np.broadcast_to(eb2[l], (P, H)) for l in range(L)], 1).astype(np.float32)
    nb2bc = np.concatenate(
        [np.broadcast_to(nb2[l], (P, H)) for l in range(L)], 1).astype(np.float32)
    nw1h = np.concatenate([nw1[l, :H] for l in range(L)], 1).astype(BF16)
    nw1a = np.concatenate([nw1[l, H:] for l in range(L)], 1).astype(BF16)
    nw2c = np.concatenate([nw2[l] for l in range(L)], 1).astype(BF16)
    nb1c = nb1.T.astype(np.float32).copy()
    lngbc = np.concatenate(
        [np.broadcast_to(ln_g[l], (P, H)) for l in range(L)], 1).astype(np.float32)
    lnbbc = np.concatenate(
        [np.broadcast_to(ln_b[l], (P, H)) for l in range(L)], 1).astype(np.float32)
    ident = np.eye(P, dtype=BF16)
    iota16 = np.tile(np.arange(P, dtype=np.int16), (P, 1))

    geom = dict(NSB_LO=NSB_LO, NSB_HI=NSB_HI, NG_LO=ng_lo, NG_HI=ng_hi,
                trivial_ln=trivial_ln)
    weights = dict(w1c4=w1c4, w1a=w1a, w1b=w1b, w2=w2, b2bc=b2bc, nb2bc=nb2bc,
                   nw1h=nw1h, nw1a=nw1a, nw2=nw2c, nb1c=nb1c,
                   lngbc=lngbc, lnbbc=lnbbc, ident=ident, iota16=iota16)
    host = dict(h1full0=h1full0.astype(BF16), target=target,
                op_w=np.asarray(inputs["op_w"], np.float32),
                op_b=np.asarray(inputs["op_b"], np.float32))
    return geom, per_core, weights, host


def _build_program(geom):
    import concourse.bass as bass
    import concourse.bacc as bacc
    import concourse.mybir as mybir
    import concourse.tile as tile

    dt = mybir.dt
    AF = mybir.ActivationFunctionType
    AF_SILU = AF.Identity if V2_SIM_SAFE else AF.Silu
    ALU = mybir.AluOpType

    NSB_LO, NSB_HI = geom["NSB_LO"], geom["NSB_HI"]
    NG_LO, NG_HI = geom["NG_LO"], geom["NG_HI"]
    NSB = NSB_LO + NSB_HI
    NG = NG_LO + NG_HI
    trivial_ln = geom["trivial_ln"]
    TROW = NCORES * NVP
    SP = V2_SINGLE_PACKET

    nc = bacc.Bacc(num_devices=NCORES, num_swdge_queues=4)

    # ---- parameters ----
    h1full0_d = nc.declare_dram_parameter("h1full0", [TROW, P], dt.bfloat16, isOutput=False)
    h1d0_d = nc.declare_dram_parameter("h1d0", [NVP + P, P], dt.bfloat16, isOutput=False)
    hbuf0_d = nc.declare_dram_parameter("hbuf0", [P, NVP], dt.bfloat16, isOutput=False)
    hT0_d = nc.declare_dram_parameter("hT0", [P, NVP], dt.bfloat16, isOutput=False)
    dstrel_d = nc.declare_dram_parameter("dstrel", [P, NSB * SUB * 4], dt.int16, isOutput=False)
    srcidx_d = nc.declare_dram_parameter("srcidx16", [P, NG * 256], dt.int16, isOutput=False)
    h1dlo_d = nc.declare_dram_parameter("h1dlo", [P, NSB_LO * 8], dt.int16, isOutput=False)
    h1dhi_d = nc.declare_dram_parameter("h1dhi", [P, NSB_HI * 8], dt.int16, isOutput=False)
    scatlo_d = nc.declare_dram_parameter("scatlo", [P, NSB_LO * 8], dt.int16, isOutput=False)
    scathi_d = nc.declare_dram_parameter("scathi", [P, NSB_HI * 8], dt.int16, isOutput=False)
    st_d = nc.declare_dram_parameter("st", [P, NSB * SB_E], dt.bfloat16, isOutput=False)
    rel4_d = nc.declare_dram_parameter("rel4", [4, NSB * SB_E], dt.bfloat16, isOutput=False)
    w1c4_d = nc.declare_dram_parameter("w1c4", [4, L * P], dt.bfloat16, isOutput=False)
    w1a_d = nc.declare_dram_parameter("w1a", [P, L * P], dt.bfloat16, isOutput=False)
    w1b_d = nc.declare_dram_parameter("w1b", [P, L * P], dt.bfloat16, isOutput=False)
    w2_d = nc.declare_dram_parameter("w2", [P, L * P], dt.bfloat16, isOutput=False)
    b2bc_d = nc.declare_dram_parameter("b2bc", [P, L * P], dt.float32, isOutput=False)
    nb2bc_d = nc.declare_dram_parameter("nb2bc", [P, L * P], dt.float32, isOutput=False)
    nw1h_d = nc.declare_dram_parameter("nw1h", [P, L * P], dt.bfloat16, isOutput=False)
    nw1a_d = nc.declare_dram_parameter("nw1a", [P, L * P], dt.bfloat16, isOutput=False)
    nw2_d = nc.declare_dram_parameter("nw2", [P, L * P], dt.bfloat16, isOutput=False)
    nb1c_d = nc.declare_dram_parameter("nb1c", [P, L], dt.float32, isOutput=False)
    lngbc_d = nc.declare_dram_parameter("lngbc", [P, L * P], dt.float32, isOutput=False)
    lnbbc_d = nc.declare_dram_parameter("lnbbc", [P, L * P], dt.float32, isOutput=False)
    ident_d = nc.declare_dram_parameter("ident", [P, P], dt.bfloat16, isOutput=False)
    iota16_d = nc.declare_dram_parameter("iota16", [P, P], dt.int16, isOutput=False)
    hout_d = nc.declare_dram_parameter("hout", [P, NVP], dt.bfloat16, isOutput=True)

    # ---- internal DRAM ----
    h1d_hbm = nc.dram_tensor("h1d_hbm", [NVP + P, P], dt.bfloat16)
    agg_hbm = nc.dram_tensor("agg_hbm", [NVP + P, P], dt.bfloat16)
    agg_zero = nc.dram_tensor("agg_zero", [NVP + P, P], dt.bfloat16)
    h1sown = nc.dram_tensor("h1sown", [NVP, P], dt.bfloat16)
    table_hi = nc.dram_tensor("table_hi", [HALF, P], dt.bfloat16)
    h1full = [nc.dram_tensor(f"h1full{l}", [TROW, P], dt.bfloat16,
                             addr_space="Shared") for l in range(1, L)]

    groups = [list(range(NCORES))]

    with tile.TileContext(nc) as tc:
        with (
            tc.tile_pool(name="const", bufs=1) as cpool,
            tc.tile_pool(name="gst", bufs=2) as gpool,      # gsT stream
            tc.tile_pool(name="sxp", bufs=2) as sxpool,     # srcidx stream
            tc.tile_pool(name="stm", bufs=2) as stpool,     # ST stream
            tc.tile_pool(name="hdb", bufs=1) as hdpool,     # h1d per-half tiles
            tc.tile_pool(name="rel", bufs=2) as rpool,
            tc.tile_pool(name="wrk", bufs=3) as wpool,      # m1s/m2s/S
            tc.tile_pool(name="ast", bufs=1) as apool,      # agg staging per half
            tc.tile_pool(name="nod", bufs=3) as npool,      # node tiles
            tc.tile_pool(name="pm1", bufs=2, space="PSUM") as pm1,
            tc.tile_pool(name="pm2", bufs=2, space="PSUM") as pm2,
            tc.tile_pool(name="pms", bufs=2, space="PSUM") as pms,  # small psum
        ):
            def cload(src, shape, dtype, tag):
                t_ = cpool.tile(shape, dtype, tag=tag)
                nc.sync.dma_start(out=t_[:], in_=src[:, :])
                return t_

            identsb = cload(ident_d, [P, P], dt.bfloat16, "ident")
            iotasb = cload(iota16_d, [P, P], dt.int16, "iota")
            dstrelsb = cload(dstrel_d, [P, NSB * SUB * 4], dt.int16, "dstrel")
            h1dlosb = cload(h1dlo_d, [P, NSB_LO * 8], dt.int16, "h1dlo")
            h1dhisb = cload(h1dhi_d, [P, NSB_HI * 8], dt.int16, "h1dhi")
            scatlosb = cload(scatlo_d, [P, NSB_LO * 8], dt.int16, "scatlo")
            scathisb = cload(scathi_d, [P, NSB_HI * 8], dt.int16, "scathi")
            w1c4sb = cload(w1c4_d, [4, L * P], dt.bfloat16, "w1c4")
            w1asb = cload(w1a_d, [P, L * P], dt.bfloat16, "w1a")
            w1bsb = cload(w1b_d, [P, L * P], dt.bfloat16, "w1b")
            w2sb = cload(w2_d, [P, L * P], dt.bfloat16, "w2")
            b2bcsb = cload(b2bc_d, [P, L * P], dt.float32, "b2bc")
            nb2bcsb = cload(nb2bc_d, [P, L * P], dt.float32, "nb2bc")
            nw1hsb = cload(nw1h_d, [P, L * P], dt.bfloat16, "nw1h")
            nw1asb = cload(nw1a_d, [P, L * P], dt.bfloat16, "nw1a")
            nw2sb = cload(nw2_d, [P, L * P], dt.bfloat16, "nw2")
            nb1csb = cload(nb1c_d, [P, L], dt.float32, "nb1c")
            if not trivial_ln:
                lngbcsb = cload(lngbc_d, [P, L * P], dt.float32, "lngbc")
                lnbbcsb = cload(lnbbc_d, [P, L * P], dt.float32, "lnbbc")

            # persistent node-state buffers (SBUF)
            hbuf = cpool.tile([P, NVP], dt.bfloat16, tag="hbuf")
            hTbuf = cpool.tile([P, NVP], dt.bfloat16, tag="hTbuf")
            h1dbuf = cpool.tile([P, NVP], dt.bfloat16, tag="h1dbuf")
            aggbuf = cpool.tile([P, NVP], dt.bfloat16, tag="aggbuf")
            xbuf = cpool.tile([P, NVP], dt.bfloat16, tag="xbuf")
            zeros = cpool.tile([P, P], dt.bfloat16, tag="zeros")
            sumbuf = cpool.tile([P, NW], dt.float32, tag="sumbuf")
            sqbuf = cpool.tile([P, NW], dt.float32, tag="sqbuf")
            mubuf = cpool.tile([P, NW], dt.float32, tag="mubuf")
            varbuf = cpool.tile([P, NW], dt.float32, tag="varbuf")
            sdbuf = cpool.tile([P, NW], dt.float32, tag="sdbuf")
            rstdb = cpool.tile([P, NW], dt.float32, tag="rstdb")
            msbuf = cpool.tile([P, NW], dt.float32, tag="msbuf")

            nc.sync.dma_start(out=hbuf[:], in_=hbuf0_d[:, :])
            nc.sync.dma_start(out=hTbuf[:], in_=hT0_d[:, :])
            nc.vector.memset(zeros[:], 0.0)
            nc.vector.memset(sumbuf[:], 0.0)
            nc.vector.memset(sqbuf[:], 1.0)
            nc.vector.memset(h1dbuf[:], 0.0)
            nc.vector.memset(xbuf[:], 0.0)
            # zero the dump rows of h1d_hbm + the agg_zero template once
            nc.sync.dma_start(out=h1d_hbm[NVP:NVP + P, :], in_=zeros[:, :])
            for zi in range(NW + 1):
                nc.sync.dma_start(out=agg_zero[zi * P:(zi + 1) * P, :],
                                  in_=zeros[:, :])

            def edge_phase(l, table_d, h1dsrc):
                lsl = slice(l * P, (l + 1) * P)
                # hi half of the gather table into its own tensor (dma_gather
                # silently ignores row offsets on in_ap)
                nc.sync.dma_start(out=table_hi[:, :], in_=table_d[HALF:, :])
                # zero agg (real + dump rows) via DRAM->DRAM template copy
                nc.sync.dma_start(out=agg_hbm[:, :], in_=agg_zero[:, :])

                # per-half h1d rows: one batched non-transpose gather each
                h1d_lo_t = hdpool.tile([P, NSB_LO * P], dt.bfloat16, tag="h1dlo_t")
                h1d_hi_t = hdpool.tile([P, NSB_HI * P], dt.bfloat16, tag="h1dhi_t")
                h1dall = {"lo": h1d_lo_t, "hi": h1d_hi_t}
                for hname, nsb_h, idxsb in (("lo", NSB_LO, h1dlosb),
                                            ("hi", NSB_HI, h1dhisb)):
                    nc.gpsimd.dma_gather(
                        out_ap=h1dall[hname][:].rearrange("p (c n) -> p c n", n=P),
                        in_ap=h1dsrc[:, :],
                        idxs_ap=idxsb[:, :],
                        num_idxs=nsb_h * P, num_idxs_reg=nsb_h * P,
                        elem_size=P, transpose=False, single_packet=SP,
                        queue_num=0)

                def load_group(g):
                    is_lo = g < NG_LO
                    tview = table_d[:, :] if is_lo else table_hi[:, :]
                    sidx = sxpool.tile([P, 256], dt.int16, tag="sidx")
                    nc.sync.dma_start(out=sidx[:],
                                      in_=srcidx_d[:, g * 256:(g + 1) * 256])
                    gsT = gpool.tile([P, G * SB_E], dt.bfloat16, tag="gsT")
                    nc.gpsimd.dma_gather(
                        out_ap=gsT[:].rearrange("p (c n) -> p c n", c=1),
                        in_ap=tview,
                        idxs_ap=sidx[:, :],
                        num_idxs=G * SB_E, num_idxs_reg=G * SB_E,
                        elem_size=P, transpose=True, single_packet=SP,
                        queue_num=0)
                    st = stpool.tile([P, G * SB_E], dt.bfloat16, tag="st")
                    nc.sync.dma_start(
                        out=st[:], in_=st_d[:, g * G * SB_E:(g + 1) * G * SB_E])
                    r4 = rpool.tile([4, G * SB_E], dt.bfloat16, tag="r4")
                    nc.sync.dma_start(
                        out=r4[:], in_=rel4_d[:, g * G * SB_E:(g + 1) * G * SB_E])
                    return gsT, st, r4

                aggst_lo = apool.tile([P, NSB_LO * P], dt.bfloat16, tag="astlo")
                aggst_hi = apool.tile([P, NSB_HI * P], dt.bfloat16, tag="asthi")
                aggst = {"lo": aggst_lo, "hi": aggst_hi}
                tiles = load_group(0)
                for g in range(NG):
                    gsT, st, r4 = tiles
                    nxt = load_group(g + 1) if g + 1 < NG else None
                    is_lo = g < NG_LO
                    hname = "lo" if is_lo else "hi"
                    h1dh = h1dall[hname]
                    boff = 0 if is_lo else NSB_LO
                    gh = g if is_lo else g - NG_LO   # group idx within half

                    for j in range(G):
                        sb = g * G + j                  # global super-block
                        sbh = gh * G + j                # super-block within half
                        aggp = pms.tile([P, P], dt.float32, tag="psA")
                        for u in range(SUB):
                            esl = slice((j * SUB + u) * BLK_E,
                                        (j * SUB + u + 1) * BLK_E)
                            m1p = pm1.tile([P, BLK_E], dt.float32, tag="m1")
                            nc.tensor.matmul(m1p[:], lhsT=h1dh[:, sbh * P:(sbh + 1) * P],
                                             rhs=st[:, esl], start=True, stop=False,
                                             skip_group_check=True)
                            nc.tensor.matmul(m1p[:], lhsT=w1c4sb[:, lsl],
                                             rhs=r4[:, esl], start=False, stop=False,
                                             skip_group_check=True)
                            nc.tensor.matmul(m1p[:], lhsT=identsb[:],
                                             rhs=gsT[:, esl], start=False, stop=True,
                                             skip_group_check=True)
                            m1s = wpool.tile([P, BLK_E], dt.bfloat16, tag="m1s")
                            nc.scalar.activation(m1s[:], m1p[:], AF_SILU)

                            m2p = pm2.tile([P, BLK_E], dt.float32, tag="m2")
                            for k in range(4):
                                ksl = slice(k * P, (k + 1) * P)
                                nc.tensor.matmul(m2p[:, ksl], lhsT=m1s[:, ksl],
                                                 rhs=w2sb[:, lsl], start=True,
                                                 stop=True, skip_group_check=True)
                            nc.vector.tensor_tensor(
                                out=m2p[:].rearrange("p (k f) -> p k f", f=P),
                                in0=m2p[:].rearrange("p (k f) -> p k f", f=P),
                                in1=b2bcsb[:, lsl].unsqueeze(1).to_broadcast([P, 4, P]),
                                op=ALU.add)
                            m2s = wpool.tile([P, BLK_E], dt.bfloat16, tag="m2s")
                            nc.scalar.activation(m2s[:], m2p[:], AF_SILU)

                            S = wpool.tile([P, 4, P], dt.bfloat16, tag="S")
                            bidx = sb * SUB + u
                            nc.vector.tensor_tensor(
                                out=S[:],
                                in0=dstrelsb[:, 4 * bidx:4 * bidx + 4].unsqueeze(2)
                                    .to_broadcast([P, 4, P]),
                                in1=iotasb[:].unsqueeze(1).to_broadcast([P, 4, P]),
                                op=ALU.is_equal)
                            for k in range(4):
                                nc.tensor.matmul(aggp[:], lhsT=S[:, k, :],
                                                 rhs=m2s[:, k * P:(k + 1) * P],
                                                 start=(u == 0 and k == 0),
                                                 stop=(u == SUB - 1 and k == 3),
                                                 skip_group_check=True)
                        nc.vector.tensor_copy(
                            aggst[hname][:, sbh * P:(sbh + 1) * P], aggp[:])

                    if (is_lo and g == NG_LO - 1) or g == NG - 1:
                        nsb_h = NSB_LO if is_lo else NSB_HI
                        scatsb = scatlosb if is_lo else scathisb
                        nc.gpsimd.dma_scatter_add(
                            out_ap=agg_hbm[:, :],
                            in_ap=aggst[hname][:].rearrange("p (c n) -> p c n", n=P),
                            idxs_ap=scatsb[:, :],
                            num_idxs=nsb_h * P, num_idxs_reg=nsb_h * P,
                            elem_size=P, single_packet=SP, queue_num=0)
                    tiles = nxt

            def node_stats_window(l, w):
                """Original per-window stats body (used for the tail window)."""
                lsl = slice(l * P, (l + 1) * P)
                cnt = min(P, NV - w * P)
                wsl = slice(w * P, w * P + P)
                csl = slice(w * P, w * P + cnt)
                tp = pms.tile([P, P], dt.bfloat16, tag="psB")
                nc.tensor.transpose(tp[:, :cnt], aggbuf[:cnt, wsl],
                                    identsb[:cnt, :cnt])
                aggt = npool.tile([P, P], dt.bfloat16, tag="aggt")
                nc.vector.tensor_copy(aggt[:, :cnt], tp[:, :cnt])
                n1p = pms.tile([P, P], dt.float32, tag="psA")
                nc.tensor.matmul(n1p[:, :cnt], lhsT=nw1hsb[:, lsl],
                                 rhs=hTbuf[:, csl], start=True, stop=False)
                nc.tensor.matmul(n1p[:, :cnt], lhsT=nw1asb[:, lsl],
                                 rhs=aggt[:, :cnt], start=False, stop=True)
                n1s = npool.tile([P, P], dt.bfloat16, tag="n1s")
                nc.scalar.activation(n1s[:, :cnt], n1p[:, :cnt], AF_SILU,
                                     bias=nb1csb[:, l:l + 1])
                n2p = pms.tile([P, P], dt.float32, tag="psB")
                nc.tensor.matmul(n2p[:cnt, :], lhsT=n1s[:, :cnt],
                                 rhs=nw2sb[:, lsl], start=True, stop=True)
                nc.vector.tensor_tensor(out=n2p[:cnt, :], in0=n2p[:cnt, :],
                                        in1=nb2bcsb[:cnt, lsl], op=ALU.add)
                nc.vector.tensor_tensor(out=n2p[:cnt, :], in0=n2p[:cnt, :],
                                        in1=hbuf[:cnt, wsl], op=ALU.add)
                nc.scalar.activation(xbuf[:cnt, wsl], n2p[:cnt, :], AF.Copy,
                                     accum_out=sumbuf[:cnt, w:w + 1])
                sqs = npool.tile([P, P], dt.float32, tag="sqs")
                nc.scalar.activation(sqs[:cnt, :], n2p[:cnt, :], AF.Square,
                                     accum_out=sqbuf[:cnt, w:w + 1])

            WB = 4                      # windows per node-phase batch
            NFB = (NV // P) // WB       # full batches (12 of 4 windows)

            def node_phase(l):
                lsl = slice(l * P, (l + 1) * P)
                last = l == L - 1
                nc.sync.dma_start(
                    out=aggbuf[:].rearrange("p (w f) -> p w f", f=P),
                    in_=agg_hbm[:NVP, :].rearrange("(p w) f -> p w f", p=P))
                for i in range(NFB):
                    bsl = slice(i * WB * P, (i + 1) * WB * P)
                    tp = pms.tile([P, WB * P], dt.bfloat16, tag="psB")
                    for k in range(WB):
                        wsl = slice((i * WB + k) * P, (i * WB + k + 1) * P)
                        nc.tensor.transpose(tp[:, k * P:(k + 1) * P],
                                            aggbuf[:, wsl], identsb[:])
                    aggt = npool.tile([P, WB * P], dt.bfloat16, tag="aggt")
                    nc.vector.tensor_copy(aggt[:], tp[:])
                    n1p = pms.tile([P, WB * P], dt.float32, tag="psA")
                    nc.tensor.matmul(n1p[:], lhsT=nw1hsb[:, lsl],
                                     rhs=hTbuf[:, bsl], start=True, stop=False)
                    nc.tensor.matmul(n1p[:], lhsT=nw1asb[:, lsl],
                                     rhs=aggt[:], start=False, stop=True)
                    n1s = npool.tile([P, WB * P], dt.bfloat16, tag="n1s")
                    nc.scalar.activation(n1s[:], n1p[:], AF_SILU,
                                         bias=nb1csb[:, l:l + 1])
                    n2p = pms.tile([P, WB * P], dt.float32, tag="psB")
                    for k in range(WB):
                        nc.tensor.matmul(n2p[:, k * P:(k + 1) * P],
                                         lhsT=n1s[:, k * P:(k + 1) * P],
                                         rhs=nw2sb[:, lsl], start=True, stop=True)
                    nc.vector.tensor_tensor(
                        out=n2p[:].rearrange("p (k f) -> p k f", f=P),
                        in0=n2p[:].rearrange("p (k f) -> p k f", f=P),
                        in1=nb2bcsb[:, lsl].unsqueeze(1).to_broadcast([P, WB, P]),
                        op=ALU.add)
                    nc.vector.tensor_tensor(out=n2p[:], in0=n2p[:],
                                            in1=hbuf[:, bsl], op=ALU.add)
                    nc.scalar.activation(xbuf[:, bsl], n2p[:], AF.Copy)
                    sqs = npool.tile([P, WB * P], dt.float32, tag="sqs")
                    nc.scalar.activation(sqs[:], n2p[:], AF.Square)
                    nc.vector.tensor_reduce(
                        out=sumbuf[:, i * WB:(i + 1) * WB],
                        in_=n2p[:].rearrange("p (k f) -> p k f", f=P),
                        axis=mybir.AxisListType.X, op=ALU.add)
                    nc.vector.tensor_reduce(
                        out=sqbuf[:, i * WB:(i + 1) * WB],
                        in_=sqs[:].rearrange("p (k f) -> p k f", f=P),
                        axis=mybir.AxisListType.X, op=ALU.add)
                for w in range(NFB * WB, NW):
                    node_stats_window(l, w)

                # batched LN stats
                nc.vector.tensor_scalar_mul(mubuf[:], sumbuf[:], 1.0 / P)
                nc.vector.tensor_tensor(out=varbuf[:], in0=mubuf[:],
                                        in1=mubuf[:], op=ALU.mult)
                nc.vector.tensor_scalar(sqbuf[:], sqbuf[:], 1.0 / P, EPS,
                                        op0=ALU.mult, op1=ALU.add)
                nc.vector.tensor_tensor(out=varbuf[:], in0=sqbuf[:],
                                        in1=varbuf[:], op=ALU.subtract)
                nc.scalar.activation(sdbuf[:], varbuf[:], AF.Sqrt)
                nc.vector.reciprocal(rstdb[:], sdbuf[:])
                nc.vector.tensor_tensor(out=msbuf[:], in0=mubuf[:],
                                        in1=rstdb[:], op=ALU.mult)

                nsl = slice((l + 1) * P, (l + 2) * P)
                if trivial_ln:
                    for i in range(NFB):
                        bsl = slice(i * WB * P, (i + 1) * WB * P)
                        wb = slice(i * WB, (i + 1) * WB)
                        nc.vector.tensor_tensor(
                            out=hbuf[:, bsl].rearrange("p (k f) -> p k f", f=P),
                            in0=xbuf[:, bsl].rearrange("p (k f) -> p k f", f=P),
                            in1=rstdb[:, wb].unsqueeze(2).to_broadcast([P, WB, P]),
                            op=ALU.mult)
                        nc.vector.tensor_tensor(
                            out=hbuf[:, bsl].rearrange("p (k f) -> p k f", f=P),
                            in0=hbuf[:, bsl].rearrange("p (k f) -> p k f", f=P),
                            in1=msbuf[:, wb].unsqueeze(2).to_broadcast([P, WB, P]),
                            op=ALU.subtract)
                        if last:
                            continue
                        tp2 = pms.tile([P, WB * P], dt.bfloat16, tag="psB")
                        for k in range(WB):
                            wsl = slice((i * WB + k) * P, (i * WB + k + 1) * P)
                            nc.tensor.transpose(tp2[:, k * P:(k + 1) * P],
                                                hbuf[:, wsl], identsb[:])
                        nc.vector.tensor_copy(hTbuf[:, bsl], tp2[:])
                        pd = pms.tile([P, WB * P], dt.float32, tag="psA")
                        ps = pms.tile([P, WB * P], dt.float32, tag="psB")
                        for k in range(WB):
                            ksl = slice(k * P, (k + 1) * P)
                            csl = slice((i * WB + k) * P, (i * WB + k + 1) * P)
                            nc.tensor.matmul(pd[:, ksl], lhsT=hTbuf[:, csl],
                                             rhs=w1asb[:, nsl], start=True,
                                             stop=True, skip_group_check=True)
                            nc.tensor.matmul(ps[:, ksl], lhsT=hTbuf[:, csl],
                                             rhs=w1bsb[:, nsl], start=True,
                                             stop=True, skip_group_check=True)
                        nc.vector.tensor_copy(h1dbuf[:, bsl], pd[:])
                        nc.vector.tensor_copy(xbuf[:, bsl], ps[:])
                    tail0 = NFB * WB
                else:
                    tail0 = 0
                for w in range(tail0, NW):
                    cnt = min(P, NV - w * P)
                    wsl = slice(w * P, w * P + P)
                    csl = slice(w * P, w * P + cnt)
                    if trivial_ln:
                        nc.vector.tensor_scalar(
                            hbuf[:cnt, wsl], xbuf[:cnt, wsl],
                            rstdb[:cnt, w:w + 1], msbuf[:cnt, w:w + 1],
                            op0=ALU.mult, op1=ALU.subtract)
                    else:
                        xn = npool.tile([P, P], dt.float32, tag="xn")
                        nc.vector.tensor_scalar(
                            xn[:cnt, :], xbuf[:cnt, wsl],
                            rstdb[:cnt, w:w + 1], msbuf[:cnt, w:w + 1],
                            op0=ALU.mult, op1=ALU.subtract)
                        nc.vector.tensor_tensor(out=xn[:cnt, :], in0=xn[:cnt, :],
                                                in1=lngbcsb[:cnt, lsl], op=ALU.mult)
                        nc.vector.tensor_tensor(out=hbuf[:cnt, wsl], in0=xn[:cnt, :],
                                                in1=lnbbcsb[:cnt, lsl], op=ALU.add)
                    if not last:
                        tp2 = pms.tile([P, P], dt.bfloat16, tag="psB")
                        nc.tensor.transpose(tp2[:, :cnt], hbuf[:cnt, wsl],
                                            identsb[:cnt, :cnt])
                        nc.vector.tensor_copy(hTbuf[:, csl], tp2[:, :cnt])
                        pd = pms.tile([P, P], dt.float32, tag="psA")
                        nc.tensor.matmul(pd[:cnt, :], lhsT=hTbuf[:, csl],
                                         rhs=w1asb[:, nsl], start=True, stop=True)
                        nc.vector.tensor_copy(h1dbuf[:cnt, wsl], pd[:cnt, :])
                        ps = pms.tile([P, P], dt.float32, tag="psB")
                        nc.tensor.matmul(ps[:cnt, :], lhsT=hTbuf[:, csl],
                                         rhs=w1bsb[:, nsl], start=True, stop=True)
                        # xbuf[w] is dead after the LN read above: reuse as
                        # the h1s staging buffer to save SBUF
                        nc.vector.tensor_copy(xbuf[:cnt, wsl], ps[:cnt, :])

                if last:
                    nc.sync.dma_start(out=hout_d[:, :], in_=hbuf[:])
                else:
                    nc.sync.dma_start(
                        out=h1d_hbm[:NVP, :].rearrange("(p w) f -> p w f", p=P),
                        in_=h1dbuf[:].rearrange("p (w f) -> p w f", f=P))
                    nc.sync.dma_start(
                        out=h1sown[:, :].rearrange("(p w) f -> p (w f)", p=P),
                        in_=xbuf[:])
                    if not V2_SKIP_CC:
                        nc.gpsimd.collective_compute(
                            "AllGather", mybir.AluOpType.bypass,
                            replica_groups=groups,
                            ins=[h1sown[:, :]],
                            outs=[h1full[l][:, :]])

            for l in range(V2_LAYERS or L):
                table = h1full0_d if l == 0 else h1full[l - 1]
                h1dsrc = h1d0_d if l == 0 else h1d_hbm
                edge_phase(l, table, h1dsrc)
                node_phase(l)

    nc.finalize()
    return nc


def _make_in_maps(geom, per_core, weights, host):
    in_maps = []
    for c in range(NCORES):
        pc = per_core[c]
        m = {
            "h1full0": host["h1full0"],
            "h1d0": pc["h1d0"], "hbuf0": pc["hbuf0"], "hT0": pc["hT0"],
            "dstrel": pc["dstrel"], "srcidx16": pc["srcidx16"],
            "h1dlo": pc["h1dlo"], "h1dhi": pc["h1dhi"],
            "scatlo": pc["scatlo"], "scathi": pc["scathi"],
            "st": pc["st"], "rel4": pc["rel4"],
        }
        m.update(weights)
        in_maps.append(m)
    return in_maps


def _postprocess(host, houts):
    hs = []
    for c in range(NCORES):
        hp = houts[c].astype(np.float32)              # [128, NVP] p-major
        hn = hp.reshape(P, NW, P).transpose(1, 0, 2).reshape(NVP, P)
        hs.append(hn[:NV])
    h = np.concatenate(hs, axis=0)
    v_pred = h @ host["op_w"] + host["op_b"]
    diff = v_pred - host["target"]
    return np.float32(np.mean(diff.astype(np.float64) ** 2))


def kernel(**inputs):
    from concourse.bass_utils import run_bass_kernel_spmd

    geom, per_core, weights, host = _preprocess(inputs)
    nc = _build_program(geom)
    in_maps = _make_in_maps(geom, per_core, weights, host)
    res = run_bass_kernel_spmd(nc, in_maps, list(range(NCORES)))
    houts = [res.results[c]["hout"] for c in range(NCORES)]
    return _postprocess(host, houts)
